# revision 7
# baseline (speedup 1.0000x reference)
"""MoE layer (E=8 experts, top-2, swiGLU) on 8 TRN2 NeuronCores.

Strategy: expert parallelism. The router (x @ Wr -> top-2 -> softmax gates)
is tiny (<0.1% of FLOPs) and is computed on host to build the dispatch:
tokens are gathered per expert into a padded capacity-C batch, one expert
per core. Each core runs the expert MLP

    y = (silu(X @ W1a + b1a) * (X @ W1b + b1b)) @ W2 + b2

entirely on device in bf16 (err ~4e-3 << 2e-2 gate; bf16 matmul streams at
the same 1 col/cycle as f32r but halves DMA traffic and enables FWL fast
weight loads). The gate scale and the scatter-add combine happen on host.

Device kernel structure (per core, SPMD — identical program, per-core data):
  Single token block: W1 streamed exactly once; xt/hT/w2 SBUF-resident.
  - xt  [P, KO1, C]  tokens, transposed, natural layout           (resident)
  - w2  [P, KO2, D]  expert W2                                    (resident)
  - hT  [P, MP, C]   swiGLU output, transposed (H on partitions)  (resident)
  - W1 streamed from HBM in [P, 2, KO1, 128] column tiles, one per mp
  GEMM1: H1T[h, t] = sum_k W1[k, h] * X[t, k]  (stationary=W1,  moving=xt)
  GEMM2: YT[d, t]  = sum_h W2[h, d] * hT[h, t] (stationary=W2t, moving=hT)
  Both GEMMs stream the token dim as the moving operand in chunks of <=512
  (PSUM bank limit) — tokens never pad to 128-tiles, so total streamed
  columns hit the MAC-count minimum. Y leaves transposed [d, t]; the host
  combine undoes it.

  Prologue: the DMA queue delivers its first bytes only at ~9 us (engine
  preamble), so mp 0 uses a RAMP chunk table [16, 64, 128, 240, 512, ...]
  with the w1[0] tile split into quarter-loads interleaved between the
  first xt pieces on the sync queue. Real matmuls start as soon as the
  first ~70 KB land and pace themselves against DMA arrival — no blind
  warmup matmuls, no >3 us PE gap (which would re-throttle the HAM clock
  gate to 1.2 GHz).
  All DMA rides the single sync (HWDGE) queue, whose in-order service is
  exactly the critical path; w2 queues behind the last w1 tile and y
  writes stream out during GEMM2.
"""

import math

import numpy as np
import ml_dtypes

import concourse.bacc as bacc
import concourse.bass as bass  # noqa: F401
import concourse.mybir as mybir
import concourse.tile as tile
from concourse.bass_utils import run_bass_kernel_spmd

P = 128
NCORES = 8

f32 = mybir.dt.float32
bf16 = mybir.dt.bfloat16
SILU = mybir.ActivationFunctionType.Silu
ADD = mybir.AluOpType.add

NP_BF16 = ml_dtypes.bfloat16


def _ramp_chunks(C):
    """mp-0 chunk table: small chunks first so compute starts while the
    DMA queue is still ramping, then 512s; remainder last."""
    ramp = [16, 64, 128, 240]
    out = []
    c0 = 0
    for r in ramp:
        if c0 + r > C:
            break
        out.append((c0, r))
        c0 += r
    while c0 + 512 <= C:
        out.append((c0, 512))
        c0 += 512
    if c0 < C:
        out.append((c0, C - c0))
    return out


def _chunks(C):
    """Steady-state chunk table: 512s, remainder last."""
    out = []
    c0 = 0
    while c0 + 512 <= C:
        out.append((c0, 512))
        c0 += 512
    if c0 < C:
        out.append((c0, C - c0))
    return out


def build_moe_expert_nc(D, H, C, has_b1=False, has_b2=False):
    """Build the SPMD per-expert kernel. D % 128 == 0, H % 128 == 0,
    C % 16 == 0 required."""
    KO1 = D // P       # k tiles of GEMM1 (contraction over D)
    MP = H // P        # hidden tiles (per swiGLU half)
    KO2 = H // P       # k tiles of GEMM2 (contraction over H)
    DP = D // P        # GEMM2 output tiles over D
    ramp = _ramp_chunks(C)
    steady = _chunks(C)

    nc = bacc.Bacc(None)
    xt_d = nc.declare_dram_parameter("xt", [P, KO1, C], bf16, isOutput=False)
    w1_d = nc.declare_dram_parameter("w1", [MP, P, 2, KO1, P], bf16, isOutput=False)
    w2_d = nc.declare_dram_parameter("w2", [P, KO2, D], bf16, isOutput=False)
    if has_b1:
        b1_d = nc.declare_dram_parameter("b1", [P, 2, MP], f32, isOutput=False)
    if has_b2:
        b2_d = nc.declare_dram_parameter("b2", [P, DP], f32, isOutput=False)
    y_d = nc.declare_dram_parameter("y", [P, DP, C], f32, isOutput=True)

    with tile.TileContext(nc) as tc:
        with (
            tc.tile_pool(name="sb", bufs=1) as sb,
            tc.tile_pool(name="ps", bufs=1, space="PSUM") as ps,
        ):
            xt_sb = sb.tile([P, KO1, C], bf16)
            w2_sb = sb.tile([P, KO2, D], bf16)
            hT = sb.tile([P, MP, C], bf16)
            if has_b1:
                b1_sb = sb.tile([P, 2, MP], f32)
            if has_b2:
                b2_sb = sb.tile([P, DP], f32)

            # PE warmup: a few matmuls on a DVE-zeroed scratch tile fill the
            # fixed PE-preamble-to-first-data window (~8 us to ~10.7 us) so
            # the HAM clock gate is already at 2.4 GHz when the first real
            # 512-wide matmuls issue.
            warm = sb.tile([P, 640], bf16)
            nc.vector.memset(warm[:].bitcast(f32), 0.0)
            for wi in range(4):
                warm_ps = ps.tile([P, 512], f32, tag=f"g1_{wi % 6}",
                                  name=f"warm_ps{wi}")
                nc.tensor.matmul(
                    warm_ps[:],
                    lhsT=warm[:, :128],
                    rhs=warm[:, 128:640],
                    start=True,
                    stop=True,
                )

            # ---- prologue loads, split across BOTH HWDGE queues (sync=SP,
            # scalar=ACT) for ~2x early bandwidth; each queue is in-order so
            # emission order is arrival order ----
            w1t0 = sb.tile([P, 2, KO1, P], bf16, tag="w1t", bufs=3, name="w1t0")
            kq = KO1 // 2
            nc.scalar.dma_start(w1t0[:, 0, :kq], w1_d[0, :, 0, :kq])
            c0r, cwr = ramp[0]
            nc.sync.dma_start(xt_sb[:, :, c0r : c0r + cwr], xt_d[:, :, c0r : c0r + cwr])
            nc.scalar.dma_start(w1t0[:, 0, kq:], w1_d[0, :, 0, kq:])
            nc.scalar.dma_start(w1t0[:, 1, :kq], w1_d[0, :, 1, :kq])
            nc.scalar.dma_start(w1t0[:, 1, kq:], w1_d[0, :, 1, kq:])
            # xt chunks alternate between the two queues
            for ci in range(1, len(ramp)):
                c0, cw = ramp[ci]
                q = nc.sync if ci % 2 == 1 else nc.scalar
                q.dma_start(xt_sb[:, :, c0 : c0 + cw], xt_d[:, :, c0 : c0 + cw])
            if has_b1:
                nc.scalar.dma_start(b1_sb[:], b1_d[:])
            if has_b2:
                nc.scalar.dma_start(b2_sb[:], b2_d[:])

            # ---- GEMM1 + swiGLU ----
            ic = 0
            for mp in range(MP):
                if mp == 0:
                    w1t = w1t0
                    chunks = ramp
                else:
                    w1t = sb.tile([P, 2, KO1, P], bf16, tag="w1t", bufs=3)
                    nc.sync.dma_start(w1t[:], w1_d[mp])
                    chunks = steady
                for c0, cw in chunks:
                    psa = ps.tile([P, 512], f32, tag=f"g1_{(2 * ic) % 6}")
                    psb = ps.tile([P, 512], f32, tag=f"g1_{(2 * ic + 1) % 6}")
                    ic += 1
                    for k in range(KO1):
                        nc.tensor.matmul(
                            psa[:, :cw],
                            lhsT=w1t[:, 0, k, :],
                            rhs=xt_sb[:, k, c0 : c0 + cw],
                            start=(k == 0),
                            stop=(k == KO1 - 1),
                        )
                    for k in range(KO1):
                        nc.tensor.matmul(
                            psb[:, :cw],
                            lhsT=w1t[:, 1, k, :],
                            rhs=xt_sb[:, k, c0 : c0 + cw],
                            start=(k == 0),
                            stop=(k == KO1 - 1),
                        )
                    # swiGLU: hT = silu(psa + b1a) * (psb + b1b)
                    sg = sb.tile([P, 512], f32, tag="sg", bufs=2)
                    if has_b1:
                        av = sb.tile([P, 512], f32, tag="av", bufs=2)
                        nc.vector.tensor_scalar_add(
                            av[:, :cw], psa[:, :cw], b1_sb[:, 0, mp : mp + 1]
                        )
                        nc.scalar.activation(sg[:, :cw], av[:, :cw], SILU)
                        bs = sb.tile([P, 512], f32, tag="bs", bufs=2)
                        nc.vector.tensor_scalar_add(
                            bs[:, :cw], psb[:, :cw], b1_sb[:, 1, mp : mp + 1]
                        )
                        nc.vector.tensor_mul(
                            hT[:, mp, c0 : c0 + cw], sg[:, :cw], bs[:, :cw]
                        )
                    else:
                        nc.scalar.activation(sg[:, :cw], psa[:, :cw], SILU)
                        nc.vector.tensor_mul(
                            hT[:, mp, c0 : c0 + cw], sg[:, :cw], psb[:, :cw]
                        )

            # w2 rides the sync queue behind the last w1 tile (arrives ~40 us
            # before GEMM2 needs it); y writes below queue after it.
            nc.sync.dma_start(w2_sb[:], w2_d[:])

            # ---- GEMM2: YT[d, t] — stationary w2 tile, moving hT ----
            iy = 0
            for dp in range(DP):
                for c0, cw in steady:
                    psy = ps.tile([P, 512], f32, tag=f"psy{iy % 2}")
                    iy += 1
                    for k in range(KO2):
                        nc.tensor.matmul(
                            psy[:, :cw],
                            lhsT=w2_sb[:, k, dp * P : (dp + 1) * P],
                            rhs=hT[:, k, c0 : c0 + cw],
                            start=(k == 0),
                            stop=(k == KO2 - 1),
                        )
                    ysb = sb.tile([P, 512], f32, tag="ysb", bufs=2)
                    if has_b2:
                        nc.vector.tensor_scalar_add(
                            ysb[:, :cw], psy[:, :cw], b2_sb[:, dp : dp + 1]
                        )
                    else:
                        nc.vector.tensor_copy(ysb[:, :cw], psy[:, :cw])
                    nc.sync.dma_start(y_d[:, dp, c0 : c0 + cw], ysb[:, :cw])
    # run_bass_via_pjrt (the axon execute path) takes a prebuilt module and
    # never finalizes it; Bacc defers register allocation to finalize().
    nc.finalize()
    return nc


def _route(x2, Wr):
    """Top-2 router, numpy fp32 (mirrors jax.lax.top_k + softmax)."""
    n = x2.shape[0]
    ar = np.arange(n)
    z = x2 @ Wr  # [N, E] fp32
    idx1 = z.argmax(axis=1)
    v1 = z[ar, idx1]
    z2 = z.copy()
    z2[ar, idx1] = -np.inf
    idx2 = z2.argmax(axis=1)
    v2 = z2[ar, idx2]
    m = np.maximum(v1, v2)
    e1 = np.exp(v1 - m)
    e2 = np.exp(v2 - m)
    s = e1 + e2
    return idx1, idx2, (e1 / s).astype(np.float32), (e2 / s).astype(np.float32)


def kernel(x, Wr, W1, b1, W2, b2):
    x = np.asarray(x, dtype=np.float32)
    Wr = np.asarray(Wr, dtype=np.float32)
    W1 = np.asarray(W1, dtype=np.float32)
    b1 = np.asarray(b1, dtype=np.float32)
    W2 = np.asarray(W2, dtype=np.float32)
    b2 = np.asarray(b2, dtype=np.float32)

    Bb, T, D = x.shape
    E, _, H2 = W1.shape
    H = H2 // 2
    N = Bb * T
    assert E == NCORES

    x2 = x.reshape(N, D)
    idx1, idx2, g1, g2 = _route(x2, Wr)

    tok = np.concatenate([np.arange(N), np.arange(N)])
    exp = np.concatenate([idx1, idx2])
    gat = np.concatenate([g1, g2])

    toks_e = [tok[exp == e] for e in range(E)]
    gats_e = [gat[exp == e] for e in range(E)]
    counts = np.array([len(t) for t in toks_e])
    C = max(512, int(math.ceil(counts.max() / 16) * 16))

    has_b1 = bool(np.any(b1))
    has_b2 = bool(np.any(b2))

    nc = build_moe_expert_nc(D, H, C, has_b1=has_b1, has_b2=has_b2)

    KO1 = D // P
    MP = H // P
    KO2 = H // P
    DP = D // P

    in_maps = []
    for e in range(E):
        ce = len(toks_e[e])
        xtf = np.zeros((D, C), dtype=NP_BF16)
        xtf[:, :ce] = x2[toks_e[e]].astype(NP_BF16).T
        xt_t = np.ascontiguousarray(
            xtf.reshape(KO1, P, C).transpose(1, 0, 2)
        )

        w1_t = np.ascontiguousarray(
            W1[e].astype(NP_BF16).reshape(KO1, P, 2, MP, P).transpose(3, 1, 2, 0, 4)
        )
        w2_t = np.ascontiguousarray(
            W2[e].astype(NP_BF16).reshape(KO2, P, D).transpose(1, 0, 2)
        )

        im = {"xt": xt_t, "w1": w1_t, "w2": w2_t}
        if has_b1:
            im["b1"] = np.ascontiguousarray(
                b1[e].reshape(2, MP, P).transpose(2, 0, 1)
            )
        if has_b2:
            im["b2"] = np.ascontiguousarray(
                b2[e].reshape(DP, P).T
            )
        in_maps.append(im)

    res = run_bass_kernel_spmd(nc, in_maps, list(range(NCORES)))

    out = np.zeros((N, D), dtype=np.float32)
    for e in range(E):
        ce = len(toks_e[e])
        # y is [P, DP, C] = YT[d % 128, d // 128, t]; undo the transpose and
        # apply the gates host-side
        yt = res.results[e]["y"]
        y2 = yt.transpose(2, 1, 0).reshape(-1, D)[:ce]
        out[toks_e[e]] += gats_e[e][:, None] * y2
    return out.reshape(Bb, T, D)


# revision 8
# speedup vs baseline: 1.0278x; 1.0278x over previous
"""MoE layer (E=8 experts, top-2, swiGLU) on 8 TRN2 NeuronCores.

Expert-PAIR parallelism: experts are sorted by routed-token count and
paired hot-with-cold; each pair is split across two cores, each core
processing half of each expert's tokens. This balances the per-core
capacity to ~(n_hot+n_cold)/2 instead of max_e(n_e) — the per-core
compute is proportional to capacity, so balancing is a direct win.
Per core: region A = [0, CA) tokens of expert a, region B = [CA, C) of
expert b (CA/CB global across cores — SPMD). Both experts' W1/W2 stream
to every core (bf16 halves the traffic; it stays far under the compute
time). Router, gates, and the scatter-add combine run on host.

Device kernel structure (per core, SPMD — identical program, per-core data):
  - xt  [P, KO1, C]  tokens, transposed, natural layout           (resident)
  - w2a/w2b [P, KO2, D]                                           (resident)
  - hT  [P, MP, C]   swiGLU output, transposed (H on partitions)  (resident)
  - W1 of both experts streamed in [P, 2, KO1, 128] tiles, one pair per mp
  GEMM1: H1T[h, t] = sum_k W1[k, h] * X[t, k]  (stationary=W1,  moving=xt)
  GEMM2: YT[d, t]  = sum_h W2[h, d] * hT[h, t] (stationary=W2t, moving=hT)
  Token dim streams as the moving operand in chunks of <=512 (PSUM bank
  limit), equalized so no chunk drops under the ~128-col dispatch floor.
  Y leaves transposed [d, t]; the host combine undoes it.

  Prologue: the DMA queues deliver first bytes only at ~9 us (engine
  preamble), so PE-idle time up to that point is absorbed by a few
  warmup matmuls on an uninitialized (output-unread) scratch tile, and
  mp 0 uses a RAMP chunk table [16, 64, 128, 240, ...] paced against DMA
  arrival. Early loads are split across BOTH HWDGE queues (sync=SP,
  scalar=ACT) for ~2x early bandwidth. No PE gap ever exceeds ~3 us, so
  the HAM clock gate stays at 2.4 GHz once warmed.
"""

import math

import numpy as np
import ml_dtypes

import concourse.bacc as bacc
import concourse.bass as bass  # noqa: F401
import concourse.mybir as mybir
import concourse.tile as tile
from concourse.bass_utils import run_bass_kernel_spmd

P = 128
NCORES = 8

f32 = mybir.dt.float32
bf16 = mybir.dt.bfloat16
SILU = mybir.ActivationFunctionType.Silu
ADD = mybir.AluOpType.add

NP_BF16 = ml_dtypes.bfloat16


def _ramp_chunks(C, base=0):
    """mp-0 chunk table: small chunks first so compute starts while the
    DMA queue is still ramping, then near-equal chunks <=512."""
    ramp = [16, 64, 128, 240]
    out = []
    c0 = 0
    for r in ramp:
        if c0 + r > C - 256 and c0 + r != C:
            break
        out.append((base + c0, r))
        c0 += r
    out += _chunks(C - c0, base + c0)
    return out


def _chunks(C, base=0):
    """Moving-dim chunks <=512 covering C, sizes equalized (multiple of 8)
    so no chunk drops under the ~128-col dispatch floor."""
    if C <= 0:
        return []
    n = (C + 511) // 512
    lo = (C // n) // 8 * 8
    out = []
    c0 = 0
    for i in range(n):
        cw = min(512, C - c0 - lo * (n - 1 - i))
        cw = cw if i < n - 1 else C - c0
        out.append((base + c0, cw))
        c0 += cw
    assert c0 == C, (C, out)
    return out


def build_moe_pair_nc(D, H, CA, CB, has_b1=False, has_b2=False):
    """Build the SPMD per-expert-pair kernel. D % 128 == 0, H % 128 == 0,
    CA % 16 == CB % 16 == 0 required."""
    C = CA + CB
    KO1 = D // P       # k tiles of GEMM1 (contraction over D)
    MP = H // P        # hidden tiles (per swiGLU half)
    KO2 = H // P       # k tiles of GEMM2 (contraction over H)
    DP = D // P        # GEMM2 output tiles over D

    rampA = _ramp_chunks(CA)
    steadyA = _chunks(CA)
    steadyB = _chunks(CB, CA)
    # mp-0 region B still rides the DMA ramp a bit: halve its first chunk
    rampB = []
    for i, (c0, cw) in enumerate(steadyB):
        if i == 0 and cw > 256:
            rampB += [(c0, 256), (c0 + 256, cw - 256)]
        else:
            rampB.append((c0, cw))

    nc = bacc.Bacc(None)
    xt_d = nc.declare_dram_parameter("xt", [P, KO1, C], bf16, isOutput=False)
    w1a_d = nc.declare_dram_parameter("w1a", [MP, P, 2, KO1, P], bf16, isOutput=False)
    w1b_d = nc.declare_dram_parameter("w1b", [MP, P, 2, KO1, P], bf16, isOutput=False)
    w2a_d = nc.declare_dram_parameter("w2a", [P, KO2, D], bf16, isOutput=False)
    w2b_d = nc.declare_dram_parameter("w2b", [P, KO2, D], bf16, isOutput=False)
    if has_b1:
        b1a_d = nc.declare_dram_parameter("b1a", [P, 2, MP], f32, isOutput=False)
        b1b_d = nc.declare_dram_parameter("b1b", [P, 2, MP], f32, isOutput=False)
    if has_b2:
        b2a_d = nc.declare_dram_parameter("b2a", [P, DP], f32, isOutput=False)
        b2b_d = nc.declare_dram_parameter("b2b", [P, DP], f32, isOutput=False)
    y_d = nc.declare_dram_parameter("y", [P, DP, C], f32, isOutput=True)

    with tile.TileContext(nc) as tc:
        with (
            tc.tile_pool(name="sb", bufs=1) as sb,
            tc.tile_pool(name="ps", bufs=1, space="PSUM") as ps,
        ):
            xt_sb = sb.tile([P, KO1, C], bf16)
            w2a_sb = sb.tile([P, KO2, D], bf16)
            w2b_sb = sb.tile([P, KO2, D], bf16)
            hT = sb.tile([P, MP, C], bf16)
            if has_b1:
                b1a_sb = sb.tile([P, 2, MP], f32)
                b1b_sb = sb.tile([P, 2, MP], f32)
            if has_b2:
                b2a_sb = sb.tile([P, DP], f32)
                b2b_sb = sb.tile([P, DP], f32)

            # PE warmup on a DVE-zeroed scratch tile: fills the fixed
            # preamble-to-first-data window AND accumulates the >=3.4 us of
            # sustained PE activity the HAM clock gate needs to unthrottle
            # (6 cold matmuls x ~0.63 us), so the first real 512-wide
            # matmuls run at 2.4 GHz.
            warm = sb.tile([P, 640], bf16)
            nc.vector.memset(warm[:].bitcast(f32), 0.0)
            for wi in range(6):
                warm_ps = ps.tile([P, 512], f32, tag=f"g1_{wi % 6}",
                                  name=f"warm_ps{wi}")
                nc.tensor.matmul(
                    warm_ps[:],
                    lhsT=warm[:, :128],
                    rhs=warm[:, 128:640],
                    start=True,
                    stop=True,
                )

            # ---- prologue loads on the single in-order sync queue, in
            # critical-path order: first matmul group's operands first ----
            w1ta0 = sb.tile([P, 2, KO1, P], bf16, tag="w1ta", bufs=3, name="w1ta0")
            w1tb0 = sb.tile([P, 2, KO1, P], bf16, tag="w1tb", bufs=3, name="w1tb0")
            kq = KO1 // 2
            nc.sync.dma_start(w1ta0[:, 0, :kq], w1a_d[0, :, 0, :kq])
            c0r, cwr = rampA[0]
            nc.sync.dma_start(xt_sb[:, :, c0r : c0r + cwr], xt_d[:, :, c0r : c0r + cwr])
            nc.sync.dma_start(w1ta0[:, 0, kq:], w1a_d[0, :, 0, kq:])
            pro_chunks = rampA[1:] + rampB
            pro_iter = iter(pro_chunks)
            for c0, cw in [next(pro_iter)]:
                nc.sync.dma_start(xt_sb[:, :, c0 : c0 + cw], xt_d[:, :, c0 : c0 + cw])
            nc.sync.dma_start(w1ta0[:, 1, :kq], w1a_d[0, :, 1, :kq])
            for c0, cw in [next(pro_iter)]:
                nc.sync.dma_start(xt_sb[:, :, c0 : c0 + cw], xt_d[:, :, c0 : c0 + cw])
            nc.sync.dma_start(w1ta0[:, 1, kq:], w1a_d[0, :, 1, kq:])
            for ci, (c0, cw) in enumerate(pro_iter):
                nc.sync.dma_start(xt_sb[:, :, c0 : c0 + cw], xt_d[:, :, c0 : c0 + cw])
                if ci == 1:
                    # w1b tile 0 lands before mp0 reaches region B
                    nc.sync.dma_start(w1tb0[:], w1b_d[0])
            if has_b1:
                nc.sync.dma_start(b1a_sb[:], b1a_d[:])
                nc.sync.dma_start(b1b_sb[:], b1b_d[:])
            if has_b2:
                nc.sync.dma_start(b2a_sb[:], b2a_d[:])
                nc.sync.dma_start(b2b_sb[:], b2b_d[:])

            def swiglu(psa, psb, cw, mp, c0, b1_sb):
                sg = sb.tile([P, 512], f32, tag="sg", bufs=2)
                if b1_sb is not None:
                    av = sb.tile([P, 512], f32, tag="av", bufs=2)
                    nc.vector.tensor_scalar_add(
                        av[:, :cw], psa[:, :cw], b1_sb[:, 0, mp : mp + 1]
                    )
                    nc.scalar.activation(sg[:, :cw], av[:, :cw], SILU)
                    bs = sb.tile([P, 512], f32, tag="bs", bufs=2)
                    nc.vector.tensor_scalar_add(
                        bs[:, :cw], psb[:, :cw], b1_sb[:, 1, mp : mp + 1]
                    )
                    nc.vector.tensor_mul(
                        hT[:, mp, c0 : c0 + cw], sg[:, :cw], bs[:, :cw]
                    )
                else:
                    nc.scalar.activation(sg[:, :cw], psa[:, :cw], SILU)
                    nc.vector.tensor_mul(
                        hT[:, mp, c0 : c0 + cw], sg[:, :cw], psb[:, :cw]
                    )

            # ---- GEMM1 + swiGLU ----
            ic = 0
            for mp in range(MP):
                if mp == 0:
                    w1ta, w1tb = w1ta0, w1tb0
                    tblA, tblB = rampA, rampB
                else:
                    w1ta = sb.tile([P, 2, KO1, P], bf16, tag="w1ta", bufs=3)
                    nc.sync.dma_start(w1ta[:], w1a_d[mp])
                    w1tb = sb.tile([P, 2, KO1, P], bf16, tag="w1tb", bufs=3)
                    nc.sync.dma_start(w1tb[:], w1b_d[mp])
                    tblA, tblB = steadyA, steadyB
                for w1t, tbl, b1s in (
                    (w1ta, tblA, b1a_sb if has_b1 else None),
                    (w1tb, tblB, b1b_sb if has_b1 else None),
                ):
                    for c0, cw in tbl:
                        psa = ps.tile([P, 512], f32, tag=f"g1_{(2 * ic) % 6}")
                        psb = ps.tile([P, 512], f32, tag=f"g1_{(2 * ic + 1) % 6}")
                        ic += 1
                        for k in range(KO1):
                            nc.tensor.matmul(
                                psa[:, :cw],
                                lhsT=w1t[:, 0, k, :],
                                rhs=xt_sb[:, k, c0 : c0 + cw],
                                start=(k == 0),
                                stop=(k == KO1 - 1),
                            )
                        for k in range(KO1):
                            nc.tensor.matmul(
                                psb[:, :cw],
                                lhsT=w1t[:, 1, k, :],
                                rhs=xt_sb[:, k, c0 : c0 + cw],
                                start=(k == 0),
                                stop=(k == KO1 - 1),
                            )
                        swiglu(psa, psb, cw, mp, c0, b1s)

            # w2 of both experts ride the queues behind the last w1 tiles
            # (arrive ~40 us before GEMM2 needs them)
            nc.sync.dma_start(w2a_sb[:], w2a_d[:])
            nc.sync.dma_start(w2b_sb[:], w2b_d[:])

            # ---- GEMM2: YT[d, t] — stationary w2 tile, moving hT ----
            iy = 0
            for dp in range(DP):
                for w2sb, tbl, b2s in (
                    (w2a_sb, steadyA, b2a_sb if has_b2 else None),
                    (w2b_sb, steadyB, b2b_sb if has_b2 else None),
                ):
                    for c0, cw in tbl:
                        psy = ps.tile([P, 512], f32, tag=f"psy{iy % 2}")
                        iy += 1
                        for k in range(KO2):
                            nc.tensor.matmul(
                                psy[:, :cw],
                                lhsT=w2sb[:, k, dp * P : (dp + 1) * P],
                                rhs=hT[:, k, c0 : c0 + cw],
                                start=(k == 0),
                                stop=(k == KO2 - 1),
                            )
                        ysb = sb.tile([P, 512], f32, tag="ysb", bufs=2)
                        if b2s is not None:
                            nc.vector.tensor_scalar_add(
                                ysb[:, :cw], psy[:, :cw], b2s[:, dp : dp + 1]
                            )
                        else:
                            nc.vector.tensor_copy(ysb[:, :cw], psy[:, :cw])
                        nc.sync.dma_start(y_d[:, dp, c0 : c0 + cw], ysb[:, :cw])
    # run_bass_via_pjrt (the axon execute path) takes a prebuilt module and
    # never finalizes it; Bacc defers register allocation to finalize().
    nc.finalize()
    return nc


def _route(x2, Wr):
    """Top-2 router, numpy fp32 (mirrors jax.lax.top_k + softmax)."""
    n = x2.shape[0]
    ar = np.arange(n)
    z = x2 @ Wr  # [N, E] fp32
    idx1 = z.argmax(axis=1)
    v1 = z[ar, idx1]
    z2 = z.copy()
    z2[ar, idx1] = -np.inf
    idx2 = z2.argmax(axis=1)
    v2 = z2[ar, idx2]
    m = np.maximum(v1, v2)
    e1 = np.exp(v1 - m)
    e2 = np.exp(v2 - m)
    s = e1 + e2
    return idx1, idx2, (e1 / s).astype(np.float32), (e2 / s).astype(np.float32)


def kernel(x, Wr, W1, b1, W2, b2):
    x = np.asarray(x, dtype=np.float32)
    Wr = np.asarray(Wr, dtype=np.float32)
    W1 = np.asarray(W1, dtype=np.float32)
    b1 = np.asarray(b1, dtype=np.float32)
    W2 = np.asarray(W2, dtype=np.float32)
    b2 = np.asarray(b2, dtype=np.float32)

    Bb, T, D = x.shape
    E, _, H2 = W1.shape
    H = H2 // 2
    N = Bb * T
    assert E == NCORES

    x2 = x.reshape(N, D)
    idx1, idx2, g1, g2 = _route(x2, Wr)

    tok = np.concatenate([np.arange(N), np.arange(N)])
    exp = np.concatenate([idx1, idx2])
    gat = np.concatenate([g1, g2])

    toks_e = [tok[exp == e] for e in range(E)]
    gats_e = [gat[exp == e] for e in range(E)]
    counts = np.array([len(t) for t in toks_e])

    # hot-with-cold expert pairing; each pair splits across two cores
    order = np.argsort(-counts)
    pairs = [(int(order[i]), int(order[E - 1 - i])) for i in range(E // 2)]
    CA = max(512, math.ceil(max((counts[a] + 1) // 2 for a, _ in pairs) / 16) * 16)
    CB = max(512, math.ceil(max((counts[b] + 1) // 2 for _, b in pairs) / 16) * 16)
    C = CA + CB

    has_b1 = bool(np.any(b1))
    has_b2 = bool(np.any(b2))

    nc = build_moe_pair_nc(D, H, CA, CB, has_b1=has_b1, has_b2=has_b2)

    KO1 = D // P
    MP = H // P
    KO2 = H // P
    DP = D // P

    # per-core token slices: core 2i gets the first halves of pair i,
    # core 2i+1 the second halves
    core_slices = []   # (a, a_toks, a_gats, b, b_toks, b_gats)
    for a, b in pairs:
        ta, ga = toks_e[a], gats_e[a]
        tb, gb = toks_e[b], gats_e[b]
        ha, hb = (len(ta) + 1) // 2, (len(tb) + 1) // 2
        core_slices.append((a, ta[:ha], ga[:ha], b, tb[:hb], gb[:hb]))
        core_slices.append((a, ta[ha:], ga[ha:], b, tb[hb:], gb[hb:]))

    w1_t = [
        np.ascontiguousarray(
            W1[e].astype(NP_BF16).reshape(KO1, P, 2, MP, P).transpose(3, 1, 2, 0, 4)
        )
        for e in range(E)
    ]
    w2_t = [
        np.ascontiguousarray(
            W2[e].astype(NP_BF16).reshape(KO2, P, D).transpose(1, 0, 2)
        )
        for e in range(E)
    ]

    in_maps = []
    for a, ta, ga, b, tb, gb in core_slices:
        xtf = np.zeros((D, C), dtype=NP_BF16)
        xtf[:, : len(ta)] = x2[ta].astype(NP_BF16).T
        xtf[:, CA : CA + len(tb)] = x2[tb].astype(NP_BF16).T
        xt_t = np.ascontiguousarray(xtf.reshape(KO1, P, C).transpose(1, 0, 2))

        im = {"xt": xt_t, "w1a": w1_t[a], "w1b": w1_t[b],
              "w2a": w2_t[a], "w2b": w2_t[b]}
        if has_b1:
            for nm, e in (("b1a", a), ("b1b", b)):
                im[nm] = np.ascontiguousarray(
                    b1[e].reshape(2, MP, P).transpose(2, 0, 1)
                )
        if has_b2:
            for nm, e in (("b2a", a), ("b2b", b)):
                im[nm] = np.ascontiguousarray(b2[e].reshape(DP, P).T)
        in_maps.append(im)

    res = run_bass_kernel_spmd(nc, in_maps, list(range(NCORES)))

    out = np.zeros((N, D), dtype=np.float32)
    for core, (a, ta, ga, b, tb, gb) in enumerate(core_slices):
        # y is [P, DP, C] = YT[d % 128, d // 128, t]; undo the transpose and
        # apply the gates host-side
        yt = res.results[core]["y"]
        y2 = yt.transpose(2, 1, 0).reshape(-1, D)
        out[ta] += ga[:, None] * y2[: len(ta)]
        out[tb] += gb[:, None] * y2[CA : CA + len(tb)]
    return out.reshape(Bb, T, D)


# revision 10
# speedup vs baseline: 1.0585x; 1.0299x over previous
"""MoE layer (E=8 experts, top-2, swiGLU) on 8 TRN2 NeuronCores.

Expert-PAIR parallelism: experts are sorted by routed-token count and
paired hot-with-cold; each pair is split across two cores, each core
processing half of each expert's tokens. This balances the per-core
capacity to ~(n_hot+n_cold)/2 instead of max_e(n_e) — the per-core
compute is proportional to capacity, so balancing is a direct win.
Per core: region A = [0, CA) tokens of expert a, region B = [CA, C) of
expert b (CA/CB global across cores — SPMD). Both experts' W1/W2 stream
to every core (bf16 halves the traffic; it stays far under the compute
time). Router, gates, and the scatter-add combine run on host.

Device kernel structure (per core, SPMD — identical program, per-core data):
  - xt  [P, KO1, C]  tokens, transposed, natural layout           (resident)
  - w2a/w2b [P, KO2, D]                                           (resident)
  - hT  [P, MP, C]   swiGLU output, transposed (H on partitions)  (resident)
  - W1 of both experts streamed in [P, 2, KO1, 128] tiles, one pair per mp
  GEMM1: H1T[h, t] = sum_k W1[k, h] * X[t, k]  (stationary=W1,  moving=xt)
  GEMM2: YT[d, t]  = sum_h W2[h, d] * hT[h, t] (stationary=W2t, moving=hT)
  Token dim streams as the moving operand in chunks of <=512 (PSUM bank
  limit), equalized so no chunk drops under the ~128-col dispatch floor.
  Y leaves transposed [d, t]; the host combine undoes it.

  Prologue: the DMA queues deliver first bytes only at ~9 us (engine
  preamble), so PE-idle time up to that point is absorbed by a few
  warmup matmuls on an uninitialized (output-unread) scratch tile, and
  mp 0 uses a RAMP chunk table [16, 64, 128, 240, ...] paced against DMA
  arrival. Early loads are split across BOTH HWDGE queues (sync=SP,
  scalar=ACT) for ~2x early bandwidth. No PE gap ever exceeds ~3 us, so
  the HAM clock gate stays at 2.4 GHz once warmed.
"""

import math

import numpy as np
import ml_dtypes

import concourse.bacc as bacc
import concourse.bass as bass  # noqa: F401
import concourse.mybir as mybir
import concourse.tile as tile
from concourse.bass_utils import run_bass_kernel_spmd

P = 128
NCORES = 8

f32 = mybir.dt.float32
bf16 = mybir.dt.bfloat16
SILU = mybir.ActivationFunctionType.Silu
ADD = mybir.AluOpType.add

NP_BF16 = ml_dtypes.bfloat16


def _ramp_chunks(C, base=0):
    """mp-0 chunk table: small chunks first so compute starts while the
    DMA queue is still ramping, then near-equal chunks <=512."""
    ramp = [16, 64, 128, 240]
    out = []
    c0 = 0
    for r in ramp:
        if c0 + r > C - 256 and c0 + r != C:
            break
        out.append((base + c0, r))
        c0 += r
    out += _chunks(C - c0, base + c0)
    return out


def _chunks(C, base=0):
    """Moving-dim chunks <=512 covering C, sizes equalized (multiple of 8)
    so no chunk drops under the ~128-col dispatch floor."""
    if C <= 0:
        return []
    n = (C + 511) // 512
    lo = (C // n) // 8 * 8
    out = []
    c0 = 0
    for i in range(n):
        cw = min(512, C - c0 - lo * (n - 1 - i))
        cw = cw if i < n - 1 else C - c0
        out.append((base + c0, cw))
        c0 += cw
    assert c0 == C, (C, out)
    return out


def build_moe_pair_nc(D, H, CA, CB, has_b1=False, has_b2=False):
    """Build the SPMD per-expert-pair kernel. D % 128 == 0, H % 128 == 0,
    CA % 16 == CB % 16 == 0 required."""
    C = CA + CB
    KO1 = D // P       # k tiles of GEMM1 (contraction over D)
    MP = H // P        # hidden tiles (per swiGLU half)
    KO2 = H // P       # k tiles of GEMM2 (contraction over H)
    DP = D // P        # GEMM2 output tiles over D

    rampA = _ramp_chunks(CA)
    steadyA = _chunks(CA)
    steadyB = _chunks(CB, CA)
    # mp-0 region B still rides the DMA ramp a bit: halve its first chunk
    rampB = []
    for i, (c0, cw) in enumerate(steadyB):
        if i == 0 and cw > 256:
            rampB += [(c0, 256), (c0 + 256, cw - 256)]
        else:
            rampB.append((c0, cw))

    nc = bacc.Bacc(None)
    xt_d = nc.declare_dram_parameter("xt", [P, KO1, C], bf16, isOutput=False)
    w1a_d = nc.declare_dram_parameter("w1a", [MP, P, 2, KO1, P], bf16, isOutput=False)
    w1b_d = nc.declare_dram_parameter("w1b", [MP, P, 2, KO1, P], bf16, isOutput=False)
    w2a_d = nc.declare_dram_parameter("w2a", [P, KO2, D], bf16, isOutput=False)
    w2b_d = nc.declare_dram_parameter("w2b", [P, KO2, D], bf16, isOutput=False)
    if has_b1:
        b1a_d = nc.declare_dram_parameter("b1a", [P, 2, MP], f32, isOutput=False)
        b1b_d = nc.declare_dram_parameter("b1b", [P, 2, MP], f32, isOutput=False)
    if has_b2:
        b2a_d = nc.declare_dram_parameter("b2a", [P, DP], f32, isOutput=False)
        b2b_d = nc.declare_dram_parameter("b2b", [P, DP], f32, isOutput=False)
    y_d = nc.declare_dram_parameter("y", [P, DP, C], f32, isOutput=True)

    with tile.TileContext(nc) as tc:
        with (
            tc.tile_pool(name="sb", bufs=1) as sb,
            tc.tile_pool(name="ps", bufs=1, space="PSUM") as ps,
        ):
            xt_sb = sb.tile([P, KO1, C], bf16)
            w2a_sb = sb.tile([P, KO2, D], bf16)
            w2b_sb = sb.tile([P, KO2, D], bf16)
            hT = sb.tile([P, MP, C], bf16)
            if has_b1:
                b1a_sb = sb.tile([P, 2, MP], f32)
                b1b_sb = sb.tile([P, 2, MP], f32)
            if has_b2:
                b2a_sb = sb.tile([P, DP], f32)
                b2b_sb = sb.tile([P, DP], f32)

            # PE warmup on a DVE-zeroed scratch tile: fills the fixed
            # preamble-to-first-data window AND accumulates the >=3.4 us of
            # sustained PE activity the HAM clock gate needs to unthrottle
            # (6 cold matmuls x ~0.63 us), so the first real 512-wide
            # matmuls run at 2.4 GHz.
            warm = sb.tile([P, 640], bf16)
            nc.vector.memset(warm[:].bitcast(f32), 0.0)
            for wi in range(6):
                warm_ps = ps.tile([P, 512], f32, tag=f"g1_{wi % 6}",
                                  name=f"warm_ps{wi}")
                nc.tensor.matmul(
                    warm_ps[:],
                    lhsT=warm[:, :128],
                    rhs=warm[:, 128:640],
                    start=True,
                    stop=True,
                )

            # ---- prologue loads on the single in-order sync queue, in
            # critical-path order: first matmul group's operands first.
            # mp 0 AND mp 1 run interleaved over the ramp chunks (below), so
            # both mps' weight tiles stream here, pieced between xt chunks.
            w1ta0 = sb.tile([P, 2, KO1, P], bf16, tag="w1ta", bufs=3, name="w1ta0")
            w1ta1 = sb.tile([P, 2, KO1, P], bf16, tag="w1ta", bufs=3, name="w1ta1")
            w1tb0 = sb.tile([P, 2, KO1, P], bf16, tag="w1tb", bufs=3, name="w1tb0")
            w1tb1 = sb.tile([P, 2, KO1, P], bf16, tag="w1tb", bufs=3, name="w1tb1")
            kq = KO1 // 2
            nc.sync.dma_start(w1ta0[:, 0, :kq], w1a_d[0, :, 0, :kq])
            c0r, cwr = rampA[0]
            nc.sync.dma_start(xt_sb[:, :, c0r : c0r + cwr], xt_d[:, :, c0r : c0r + cwr])
            nc.sync.dma_start(w1ta0[:, 0, kq:], w1a_d[0, :, 0, kq:])
            pro_chunks = rampA[1:] + rampB
            pro_iter = iter(pro_chunks)
            for c0, cw in [next(pro_iter)]:
                nc.sync.dma_start(xt_sb[:, :, c0 : c0 + cw], xt_d[:, :, c0 : c0 + cw])
            nc.sync.dma_start(w1ta1[:, 0], w1a_d[1, :, 0])
            for c0, cw in [next(pro_iter)]:
                nc.sync.dma_start(xt_sb[:, :, c0 : c0 + cw], xt_d[:, :, c0 : c0 + cw])
            nc.sync.dma_start(w1ta0[:, 1], w1a_d[0, :, 1])
            nc.sync.dma_start(w1ta1[:, 1], w1a_d[1, :, 1])
            for ci, (c0, cw) in enumerate(pro_iter):
                nc.sync.dma_start(xt_sb[:, :, c0 : c0 + cw], xt_d[:, :, c0 : c0 + cw])
                if ci == 0:
                    # both w1b tiles land before the interleave reaches B
                    nc.sync.dma_start(w1tb0[:], w1b_d[0])
                if ci == 1:
                    nc.sync.dma_start(w1tb1[:], w1b_d[1])
            if has_b1:
                nc.sync.dma_start(b1a_sb[:], b1a_d[:])
                nc.sync.dma_start(b1b_sb[:], b1b_d[:])
            if has_b2:
                nc.sync.dma_start(b2a_sb[:], b2a_d[:])
                nc.sync.dma_start(b2b_sb[:], b2b_d[:])

            def swiglu(psa, psb, cw, mp, c0, b1_sb):
                sg = sb.tile([P, 512], f32, tag="sg", bufs=2)
                if b1_sb is not None:
                    av = sb.tile([P, 512], f32, tag="av", bufs=2)
                    nc.vector.tensor_scalar_add(
                        av[:, :cw], psa[:, :cw], b1_sb[:, 0, mp : mp + 1]
                    )
                    nc.scalar.activation(sg[:, :cw], av[:, :cw], SILU)
                    bs = sb.tile([P, 512], f32, tag="bs", bufs=2)
                    nc.vector.tensor_scalar_add(
                        bs[:, :cw], psb[:, :cw], b1_sb[:, 1, mp : mp + 1]
                    )
                    nc.vector.tensor_mul(
                        hT[:, mp, c0 : c0 + cw], sg[:, :cw], bs[:, :cw]
                    )
                else:
                    nc.scalar.activation(sg[:, :cw], psa[:, :cw], SILU)
                    nc.vector.tensor_mul(
                        hT[:, mp, c0 : c0 + cw], sg[:, :cw], psb[:, :cw]
                    )

            # ---- GEMM1 + swiGLU ----
            ic = 0

            def g1_group(w1t, mp, c0, cw, b1s):
                nonlocal ic
                psa = ps.tile([P, 512], f32, tag=f"g1_{(2 * ic) % 6}")
                psb = ps.tile([P, 512], f32, tag=f"g1_{(2 * ic + 1) % 6}")
                ic += 1
                for k in range(KO1):
                    nc.tensor.matmul(
                        psa[:, :cw],
                        lhsT=w1t[:, 0, k, :],
                        rhs=xt_sb[:, k, c0 : c0 + cw],
                        start=(k == 0),
                        stop=(k == KO1 - 1),
                    )
                for k in range(KO1):
                    nc.tensor.matmul(
                        psb[:, :cw],
                        lhsT=w1t[:, 1, k, :],
                        rhs=xt_sb[:, k, c0 : c0 + cw],
                        start=(k == 0),
                        stop=(k == KO1 - 1),
                    )
                swiglu(psa, psb, cw, mp, c0, b1s)

            b1a = b1a_sb if has_b1 else None
            b1b = b1b_sb if has_b1 else None

            # mps 0 and 1 interleave over the ramp chunks: in the DMA-ramp
            # window, doubling the compute per arrived xt byte keeps the PE
            # fed (no >3 us gap -> no HAM re-throttle) at zero added work.
            for c0, cw in rampA:
                g1_group(w1ta0, 0, c0, cw, b1a)
                g1_group(w1ta1, 1, c0, cw, b1a)
            for c0, cw in rampB:
                g1_group(w1tb0, 0, c0, cw, b1b)
                g1_group(w1tb1, 1, c0, cw, b1b)

            for mp in range(2, MP):
                w1ta = sb.tile([P, 2, KO1, P], bf16, tag="w1ta", bufs=3)
                nc.sync.dma_start(w1ta[:], w1a_d[mp])
                w1tb = sb.tile([P, 2, KO1, P], bf16, tag="w1tb", bufs=3)
                nc.sync.dma_start(w1tb[:], w1b_d[mp])
                for c0, cw in steadyA:
                    g1_group(w1ta, mp, c0, cw, b1a)
                for c0, cw in steadyB:
                    g1_group(w1tb, mp, c0, cw, b1b)

            # w2 of both experts ride the queues behind the last w1 tiles
            # (arrive ~40 us before GEMM2 needs them)
            nc.sync.dma_start(w2a_sb[:], w2a_d[:])
            nc.sync.dma_start(w2b_sb[:], w2b_d[:])

            # ---- GEMM2: YT[d, t] — stationary w2 tile, moving hT ----
            iy = 0
            for dp in range(DP):
                for w2sb, tbl, b2s in (
                    (w2a_sb, steadyA, b2a_sb if has_b2 else None),
                    (w2b_sb, steadyB, b2b_sb if has_b2 else None),
                ):
                    for c0, cw in tbl:
                        psy = ps.tile([P, 512], f32, tag=f"psy{iy % 2}")
                        iy += 1
                        for k in range(KO2):
                            nc.tensor.matmul(
                                psy[:, :cw],
                                lhsT=w2sb[:, k, dp * P : (dp + 1) * P],
                                rhs=hT[:, k, c0 : c0 + cw],
                                start=(k == 0),
                                stop=(k == KO2 - 1),
                            )
                        ysb = sb.tile([P, 512], f32, tag="ysb", bufs=2)
                        if b2s is not None:
                            nc.vector.tensor_scalar_add(
                                ysb[:, :cw], psy[:, :cw], b2s[:, dp : dp + 1]
                            )
                        else:
                            nc.vector.tensor_copy(ysb[:, :cw], psy[:, :cw])
                        nc.sync.dma_start(y_d[:, dp, c0 : c0 + cw], ysb[:, :cw])
    # run_bass_via_pjrt (the axon execute path) takes a prebuilt module and
    # never finalizes it; Bacc defers register allocation to finalize().
    nc.finalize()
    return nc


def _route(x2, Wr):
    """Top-2 router, numpy fp32 (mirrors jax.lax.top_k + softmax)."""
    n = x2.shape[0]
    ar = np.arange(n)
    z = x2 @ Wr  # [N, E] fp32
    idx1 = z.argmax(axis=1)
    v1 = z[ar, idx1]
    z2 = z.copy()
    z2[ar, idx1] = -np.inf
    idx2 = z2.argmax(axis=1)
    v2 = z2[ar, idx2]
    m = np.maximum(v1, v2)
    e1 = np.exp(v1 - m)
    e2 = np.exp(v2 - m)
    s = e1 + e2
    return idx1, idx2, (e1 / s).astype(np.float32), (e2 / s).astype(np.float32)


def kernel(x, Wr, W1, b1, W2, b2):
    x = np.asarray(x, dtype=np.float32)
    Wr = np.asarray(Wr, dtype=np.float32)
    W1 = np.asarray(W1, dtype=np.float32)
    b1 = np.asarray(b1, dtype=np.float32)
    W2 = np.asarray(W2, dtype=np.float32)
    b2 = np.asarray(b2, dtype=np.float32)

    Bb, T, D = x.shape
    E, _, H2 = W1.shape
    H = H2 // 2
    N = Bb * T
    assert E == NCORES

    x2 = x.reshape(N, D)
    idx1, idx2, g1, g2 = _route(x2, Wr)

    tok = np.concatenate([np.arange(N), np.arange(N)])
    exp = np.concatenate([idx1, idx2])
    gat = np.concatenate([g1, g2])

    toks_e = [tok[exp == e] for e in range(E)]
    gats_e = [gat[exp == e] for e in range(E)]
    counts = np.array([len(t) for t in toks_e])

    # hot-with-cold expert pairing; each pair splits across two cores
    order = np.argsort(-counts)
    pairs = [(int(order[i]), int(order[E - 1 - i])) for i in range(E // 2)]
    CA = max(512, math.ceil(max((counts[a] + 1) // 2 for a, _ in pairs) / 16) * 16)
    CB = max(512, math.ceil(max((counts[b] + 1) // 2 for _, b in pairs) / 16) * 16)
    C = CA + CB

    has_b1 = bool(np.any(b1))
    has_b2 = bool(np.any(b2))

    nc = build_moe_pair_nc(D, H, CA, CB, has_b1=has_b1, has_b2=has_b2)

    KO1 = D // P
    MP = H // P
    KO2 = H // P
    DP = D // P

    # per-core token slices: core 2i gets the first halves of pair i,
    # core 2i+1 the second halves
    core_slices = []   # (a, a_toks, a_gats, b, b_toks, b_gats)
    for a, b in pairs:
        ta, ga = toks_e[a], gats_e[a]
        tb, gb = toks_e[b], gats_e[b]
        ha, hb = (len(ta) + 1) // 2, (len(tb) + 1) // 2
        core_slices.append((a, ta[:ha], ga[:ha], b, tb[:hb], gb[:hb]))
        core_slices.append((a, ta[ha:], ga[ha:], b, tb[hb:], gb[hb:]))

    w1_t = [
        np.ascontiguousarray(
            W1[e].astype(NP_BF16).reshape(KO1, P, 2, MP, P).transpose(3, 1, 2, 0, 4)
        )
        for e in range(E)
    ]
    w2_t = [
        np.ascontiguousarray(
            W2[e].astype(NP_BF16).reshape(KO2, P, D).transpose(1, 0, 2)
        )
        for e in range(E)
    ]

    in_maps = []
    for a, ta, ga, b, tb, gb in core_slices:
        xtf = np.zeros((D, C), dtype=NP_BF16)
        xtf[:, : len(ta)] = x2[ta].astype(NP_BF16).T
        xtf[:, CA : CA + len(tb)] = x2[tb].astype(NP_BF16).T
        xt_t = np.ascontiguousarray(xtf.reshape(KO1, P, C).transpose(1, 0, 2))

        im = {"xt": xt_t, "w1a": w1_t[a], "w1b": w1_t[b],
              "w2a": w2_t[a], "w2b": w2_t[b]}
        if has_b1:
            for nm, e in (("b1a", a), ("b1b", b)):
                im[nm] = np.ascontiguousarray(
                    b1[e].reshape(2, MP, P).transpose(2, 0, 1)
                )
        if has_b2:
            for nm, e in (("b2a", a), ("b2b", b)):
                im[nm] = np.ascontiguousarray(b2[e].reshape(DP, P).T)
        in_maps.append(im)

    res = run_bass_kernel_spmd(nc, in_maps, list(range(NCORES)))

    out = np.zeros((N, D), dtype=np.float32)
    for core, (a, ta, ga, b, tb, gb) in enumerate(core_slices):
        # y is [P, DP, C] = YT[d % 128, d // 128, t]; undo the transpose and
        # apply the gates host-side
        yt = res.results[core]["y"]
        y2 = yt.transpose(2, 1, 0).reshape(-1, D)
        out[ta] += ga[:, None] * y2[: len(ta)]
        out[tb] += gb[:, None] * y2[CA : CA + len(tb)]
    return out.reshape(Bb, T, D)


# revision 11
# speedup vs baseline: 1.0674x; 1.0084x over previous
"""MoE layer (E=8 experts, top-2, swiGLU) on 8 TRN2 NeuronCores.

Expert-PAIR parallelism: experts are sorted by routed-token count and
paired hot-with-cold; each pair is split across two cores, each core
processing half of each expert's tokens. This balances the per-core
capacity to ~(n_hot+n_cold)/2 instead of max_e(n_e) — the per-core
compute is proportional to capacity, so balancing is a direct win.
Per core: region A = [0, CA) tokens of expert a, region B = [CA, C) of
expert b (CA/CB global across cores — SPMD). Both experts' W1/W2 stream
to every core (bf16 halves the traffic; it stays far under the compute
time). Router, gates, and the scatter-add combine run on host.

Device kernel structure (per core, SPMD — identical program, per-core data):
  - xt  [P, KO1, C]  tokens, transposed, natural layout           (resident)
  - w2a/w2b [P, KO2, D]                                           (resident)
  - hT  [P, MP, C]   swiGLU output, transposed (H on partitions)  (resident)
  - W1 of both experts streamed in [P, 2, KO1, 128] tiles, one pair per mp
  GEMM1: H1T[h, t] = sum_k W1[k, h] * X[t, k]  (stationary=W1,  moving=xt)
  GEMM2: YT[d, t]  = sum_h W2[h, d] * hT[h, t] (stationary=W2t, moving=hT)
  Token dim streams as the moving operand in chunks of <=512 (PSUM bank
  limit), equalized so no chunk drops under the ~128-col dispatch floor.
  Y leaves transposed [d, t]; the host combine undoes it.

  Prologue: the DMA queues deliver first bytes only at ~9 us (engine
  preamble), so PE-idle time up to that point is absorbed by a few
  warmup matmuls on an uninitialized (output-unread) scratch tile, and
  mp 0 uses a RAMP chunk table [16, 64, 128, 240, ...] paced against DMA
  arrival. Early loads are split across BOTH HWDGE queues (sync=SP,
  scalar=ACT) for ~2x early bandwidth. No PE gap ever exceeds ~3 us, so
  the HAM clock gate stays at 2.4 GHz once warmed.
"""

import math

import numpy as np
import ml_dtypes

import concourse.bacc as bacc
import concourse.bass as bass  # noqa: F401
import concourse.mybir as mybir
import concourse.tile as tile
from concourse.bass_utils import run_bass_kernel_spmd

P = 128
NCORES = 8

f32 = mybir.dt.float32
bf16 = mybir.dt.bfloat16
SILU = mybir.ActivationFunctionType.Silu
ADD = mybir.AluOpType.add

NP_BF16 = ml_dtypes.bfloat16


def _ramp_chunks(C, base=0):
    """mp-0 chunk table: small chunks first so compute starts while the
    DMA queue is still ramping, then near-equal chunks <=512."""
    ramp = [16, 64, 128, 240]
    out = []
    c0 = 0
    for r in ramp:
        if c0 + r > C - 256 and c0 + r != C:
            break
        out.append((base + c0, r))
        c0 += r
    out += _chunks(C - c0, base + c0)
    return out


def _chunks(C, base=0):
    """Moving-dim chunks <=512 covering C, sizes equalized (multiple of 8)
    so no chunk drops under the ~128-col dispatch floor."""
    if C <= 0:
        return []
    n = (C + 511) // 512
    lo = (C // n) // 8 * 8
    out = []
    c0 = 0
    for i in range(n):
        cw = min(512, C - c0 - lo * (n - 1 - i))
        cw = cw if i < n - 1 else C - c0
        out.append((base + c0, cw))
        c0 += cw
    assert c0 == C, (C, out)
    return out


def build_moe_pair_nc(D, H, CA, CB, has_b1=False, has_b2=False):
    """Build the SPMD per-expert-pair kernel. D % 128 == 0, H % 128 == 0,
    CA % 16 == CB % 16 == 0 required."""
    C = CA + CB
    KO1 = D // P       # k tiles of GEMM1 (contraction over D)
    MP = H // P        # hidden tiles (per swiGLU half)
    KO2 = H // P       # k tiles of GEMM2 (contraction over H)
    DP = D // P        # GEMM2 output tiles over D

    rampA = _ramp_chunks(CA)
    steadyA = _chunks(CA)
    steadyB = _chunks(CB, CA)
    # mp-0 region B still rides the DMA ramp a bit: halve its first chunk
    rampB = []
    for i, (c0, cw) in enumerate(steadyB):
        if i == 0 and cw > 256:
            rampB += [(c0, 256), (c0 + 256, cw - 256)]
        else:
            rampB.append((c0, cw))

    nc = bacc.Bacc(None)
    xt_d = nc.declare_dram_parameter("xt", [P, KO1, C], bf16, isOutput=False)
    w1a_d = nc.declare_dram_parameter("w1a", [MP, P, 2, KO1, P], bf16, isOutput=False)
    w1b_d = nc.declare_dram_parameter("w1b", [MP, P, 2, KO1, P], bf16, isOutput=False)
    w2a_d = nc.declare_dram_parameter("w2a", [P, KO2, D], bf16, isOutput=False)
    w2b_d = nc.declare_dram_parameter("w2b", [P, KO2, D], bf16, isOutput=False)
    if has_b1:
        b1a_d = nc.declare_dram_parameter("b1a", [P, 2, MP], f32, isOutput=False)
        b1b_d = nc.declare_dram_parameter("b1b", [P, 2, MP], f32, isOutput=False)
    if has_b2:
        b2a_d = nc.declare_dram_parameter("b2a", [P, DP], f32, isOutput=False)
        b2b_d = nc.declare_dram_parameter("b2b", [P, DP], f32, isOutput=False)
    y_d = nc.declare_dram_parameter("y", [P, DP, C], f32, isOutput=True)

    with tile.TileContext(nc) as tc:
        with (
            tc.tile_pool(name="sb", bufs=1) as sb,
            tc.tile_pool(name="ps", bufs=1, space="PSUM") as ps,
        ):
            xt_sb = sb.tile([P, KO1, C], bf16)
            w2a_sb = sb.tile([P, KO2, D], bf16)
            w2b_sb = sb.tile([P, KO2, D], bf16)
            hT = sb.tile([P, MP, C], bf16)
            if has_b1:
                b1a_sb = sb.tile([P, 2, MP], f32)
                b1b_sb = sb.tile([P, 2, MP], f32)
            if has_b2:
                b2a_sb = sb.tile([P, DP], f32)
                b2b_sb = sb.tile([P, DP], f32)

            # PE warmup on a DVE-zeroed scratch tile: fills the fixed
            # preamble-to-first-data window AND accumulates the >=3.4 us of
            # sustained PE activity the HAM clock gate needs to unthrottle
            # (6 cold matmuls x ~0.63 us), so the first real 512-wide
            # matmuls run at 2.4 GHz.
            warm = sb.tile([P, 640], bf16)
            nc.vector.memset(warm[:].bitcast(f32), 0.0)
            for wi in range(6):
                warm_ps = ps.tile([P, 512], f32, tag=f"g1_{wi % 6}",
                                  name=f"warm_ps{wi}")
                nc.tensor.matmul(
                    warm_ps[:],
                    lhsT=warm[:, :128],
                    rhs=warm[:, 128:640],
                    start=True,
                    stop=True,
                )

            # ---- prologue loads on the single in-order sync queue, in
            # critical-path order: first matmul group's operands first.
            # mp 0 AND mp 1 run interleaved over the ramp chunks (below), so
            # both mps' weight tiles stream here, pieced between xt chunks.
            w1ta0 = sb.tile([P, 2, KO1, P], bf16, tag="w1ta", bufs=3, name="w1ta0")
            w1ta1 = sb.tile([P, 2, KO1, P], bf16, tag="w1ta", bufs=3, name="w1ta1")
            w1tb0 = sb.tile([P, 2, KO1, P], bf16, tag="w1tb", bufs=3, name="w1tb0")
            w1tb1 = sb.tile([P, 2, KO1, P], bf16, tag="w1tb", bufs=3, name="w1tb1")
            kq = KO1 // 2
            nc.sync.dma_start(w1ta0[:, 0, :kq], w1a_d[0, :, 0, :kq])
            c0r, cwr = rampA[0]
            nc.sync.dma_start(xt_sb[:, :, c0r : c0r + cwr], xt_d[:, :, c0r : c0r + cwr])
            nc.sync.dma_start(w1ta0[:, 0, kq:], w1a_d[0, :, 0, kq:])
            pro_chunks = rampA[1:] + rampB
            pro_iter = iter(pro_chunks)
            for c0, cw in [next(pro_iter)]:
                nc.sync.dma_start(xt_sb[:, :, c0 : c0 + cw], xt_d[:, :, c0 : c0 + cw])
            nc.sync.dma_start(w1ta1[:, 0], w1a_d[1, :, 0])
            for c0, cw in [next(pro_iter)]:
                nc.sync.dma_start(xt_sb[:, :, c0 : c0 + cw], xt_d[:, :, c0 : c0 + cw])
            nc.sync.dma_start(w1ta0[:, 1], w1a_d[0, :, 1])
            nc.sync.dma_start(w1ta1[:, 1], w1a_d[1, :, 1])
            for ci, (c0, cw) in enumerate(pro_iter):
                nc.sync.dma_start(xt_sb[:, :, c0 : c0 + cw], xt_d[:, :, c0 : c0 + cw])
                if ci == 0:
                    # both w1b tiles land before the interleave reaches B
                    nc.sync.dma_start(w1tb0[:], w1b_d[0])
                if ci == 1:
                    nc.sync.dma_start(w1tb1[:], w1b_d[1])
            if has_b1:
                nc.sync.dma_start(b1a_sb[:], b1a_d[:])
                nc.sync.dma_start(b1b_sb[:], b1b_d[:])
            if has_b2:
                nc.sync.dma_start(b2a_sb[:], b2a_d[:])
                nc.sync.dma_start(b2b_sb[:], b2b_d[:])

            def swiglu(psa, psb, cw, mp, c0, b1_sb):
                sg = sb.tile([P, 512], f32, tag="sg", bufs=2)
                if b1_sb is not None:
                    av = sb.tile([P, 512], f32, tag="av", bufs=2)
                    nc.vector.tensor_scalar_add(
                        av[:, :cw], psa[:, :cw], b1_sb[:, 0, mp : mp + 1]
                    )
                    nc.scalar.activation(sg[:, :cw], av[:, :cw], SILU)
                    bs = sb.tile([P, 512], f32, tag="bs", bufs=2)
                    nc.vector.tensor_scalar_add(
                        bs[:, :cw], psb[:, :cw], b1_sb[:, 1, mp : mp + 1]
                    )
                    nc.vector.tensor_mul(
                        hT[:, mp, c0 : c0 + cw], sg[:, :cw], bs[:, :cw]
                    )
                else:
                    nc.scalar.activation(sg[:, :cw], psa[:, :cw], SILU)
                    nc.vector.tensor_mul(
                        hT[:, mp, c0 : c0 + cw], sg[:, :cw], psb[:, :cw]
                    )

            # ---- GEMM1 + swiGLU ----
            ic = 0

            def g1_group(w1t, mp, c0, cw, b1s):
                nonlocal ic
                psa = ps.tile([P, 512], f32, tag=f"g1_{(2 * ic) % 6}")
                psb = ps.tile([P, 512], f32, tag=f"g1_{(2 * ic + 1) % 6}")
                ic += 1
                for k in range(KO1):
                    nc.tensor.matmul(
                        psa[:, :cw],
                        lhsT=w1t[:, 0, k, :],
                        rhs=xt_sb[:, k, c0 : c0 + cw],
                        start=(k == 0),
                        stop=(k == KO1 - 1),
                    )
                for k in range(KO1):
                    nc.tensor.matmul(
                        psb[:, :cw],
                        lhsT=w1t[:, 1, k, :],
                        rhs=xt_sb[:, k, c0 : c0 + cw],
                        start=(k == 0),
                        stop=(k == KO1 - 1),
                    )
                swiglu(psa, psb, cw, mp, c0, b1s)

            b1a = b1a_sb if has_b1 else None
            b1b = b1b_sb if has_b1 else None

            # mps 0 and 1 interleave over the ramp chunks: in the DMA-ramp
            # window, doubling the compute per arrived xt byte keeps the PE
            # fed (no >3 us gap -> no HAM re-throttle) at zero added work.
            for c0, cw in rampA:
                g1_group(w1ta0, 0, c0, cw, b1a)
                g1_group(w1ta1, 1, c0, cw, b1a)
            for c0, cw in rampB:
                g1_group(w1tb0, 0, c0, cw, b1b)
                g1_group(w1tb1, 1, c0, cw, b1b)

            for mp in range(2, MP):
                w1ta = sb.tile([P, 2, KO1, P], bf16, tag="w1ta", bufs=3)
                nc.sync.dma_start(w1ta[:], w1a_d[mp])
                w1tb = sb.tile([P, 2, KO1, P], bf16, tag="w1tb", bufs=3)
                nc.sync.dma_start(w1tb[:], w1b_d[mp])
                for c0, cw in steadyA:
                    g1_group(w1ta, mp, c0, cw, b1a)
                for c0, cw in steadyB:
                    g1_group(w1tb, mp, c0, cw, b1b)

            # w2 of both experts ride the queues behind the last w1 tiles
            # (arrive ~40 us before GEMM2 needs them)
            nc.sync.dma_start(w2a_sb[:], w2a_d[:])
            nc.sync.dma_start(w2b_sb[:], w2b_d[:])

            # ---- GEMM2: YT[d, t] — stationary w2 tile, moving hT ----
            iy = 0
            for dp in range(DP):
                for w2sb, tbl, b2s in (
                    (w2a_sb, steadyA, b2a_sb if has_b2 else None),
                    (w2b_sb, steadyB, b2b_sb if has_b2 else None),
                ):
                    for c0, cw in tbl:
                        psy = ps.tile([P, 512], f32, tag=f"psy{iy % 2}")
                        iy += 1
                        for k in range(KO2):
                            nc.tensor.matmul(
                                psy[:, :cw],
                                lhsT=w2sb[:, k, dp * P : (dp + 1) * P],
                                rhs=hT[:, k, c0 : c0 + cw],
                                start=(k == 0),
                                stop=(k == KO2 - 1),
                            )
                        ysb = sb.tile([P, 512], f32, tag="ysb", bufs=2)
                        if b2s is not None:
                            nc.vector.tensor_scalar_add(
                                ysb[:, :cw], psy[:, :cw], b2s[:, dp : dp + 1]
                            )
                        else:
                            nc.vector.tensor_copy(ysb[:, :cw], psy[:, :cw])
                        nc.sync.dma_start(y_d[:, dp, c0 : c0 + cw], ysb[:, :cw])
    # run_bass_via_pjrt (the axon execute path) takes a prebuilt module and
    # never finalizes it; Bacc defers register allocation to finalize().
    nc.finalize()
    return nc


def _route(x2, Wr):
    """Top-2 router, numpy fp32 (mirrors jax.lax.top_k + softmax)."""
    n = x2.shape[0]
    ar = np.arange(n)
    z = x2 @ Wr  # [N, E] fp32
    idx1 = z.argmax(axis=1)
    v1 = z[ar, idx1]
    z2 = z.copy()
    z2[ar, idx1] = -np.inf
    idx2 = z2.argmax(axis=1)
    v2 = z2[ar, idx2]
    m = np.maximum(v1, v2)
    e1 = np.exp(v1 - m)
    e2 = np.exp(v2 - m)
    s = e1 + e2
    return idx1, idx2, (e1 / s).astype(np.float32), (e2 / s).astype(np.float32)


def kernel(x, Wr, W1, b1, W2, b2):
    x = np.asarray(x, dtype=np.float32)
    Wr = np.asarray(Wr, dtype=np.float32)
    W1 = np.asarray(W1, dtype=np.float32)
    b1 = np.asarray(b1, dtype=np.float32)
    W2 = np.asarray(W2, dtype=np.float32)
    b2 = np.asarray(b2, dtype=np.float32)

    Bb, T, D = x.shape
    E, _, H2 = W1.shape
    H = H2 // 2
    N = Bb * T
    assert E == NCORES

    x2 = x.reshape(N, D)
    idx1, idx2, g1, g2 = _route(x2, Wr)

    tok = np.concatenate([np.arange(N), np.arange(N)])
    exp = np.concatenate([idx1, idx2])
    gat = np.concatenate([g1, g2])

    toks_e = [tok[exp == e] for e in range(E)]
    gats_e = [gat[exp == e] for e in range(E)]
    counts = np.array([len(t) for t in toks_e])

    # hot-with-cold expert pairing; each pair splits across two cores
    order = np.argsort(-counts)
    pairs = [(int(order[i]), int(order[E - 1 - i])) for i in range(E // 2)]
    CA = max(512, math.ceil(max((counts[a] + 1) // 2 for a, _ in pairs) / 4) * 4)
    CB = max(512, math.ceil(max((counts[b] + 1) // 2 for _, b in pairs) / 4) * 4)
    C = CA + CB

    has_b1 = bool(np.any(b1))
    has_b2 = bool(np.any(b2))

    nc = build_moe_pair_nc(D, H, CA, CB, has_b1=has_b1, has_b2=has_b2)

    KO1 = D // P
    MP = H // P
    KO2 = H // P
    DP = D // P

    # per-core token slices: core 2i gets the first halves of pair i,
    # core 2i+1 the second halves
    core_slices = []   # (a, a_toks, a_gats, b, b_toks, b_gats)
    for a, b in pairs:
        ta, ga = toks_e[a], gats_e[a]
        tb, gb = toks_e[b], gats_e[b]
        ha, hb = (len(ta) + 1) // 2, (len(tb) + 1) // 2
        core_slices.append((a, ta[:ha], ga[:ha], b, tb[:hb], gb[:hb]))
        core_slices.append((a, ta[ha:], ga[ha:], b, tb[hb:], gb[hb:]))

    w1_t = [
        np.ascontiguousarray(
            W1[e].astype(NP_BF16).reshape(KO1, P, 2, MP, P).transpose(3, 1, 2, 0, 4)
        )
        for e in range(E)
    ]
    w2_t = [
        np.ascontiguousarray(
            W2[e].astype(NP_BF16).reshape(KO2, P, D).transpose(1, 0, 2)
        )
        for e in range(E)
    ]

    in_maps = []
    for a, ta, ga, b, tb, gb in core_slices:
        xtf = np.zeros((D, C), dtype=NP_BF16)
        xtf[:, : len(ta)] = x2[ta].astype(NP_BF16).T
        xtf[:, CA : CA + len(tb)] = x2[tb].astype(NP_BF16).T
        xt_t = np.ascontiguousarray(xtf.reshape(KO1, P, C).transpose(1, 0, 2))

        im = {"xt": xt_t, "w1a": w1_t[a], "w1b": w1_t[b],
              "w2a": w2_t[a], "w2b": w2_t[b]}
        if has_b1:
            for nm, e in (("b1a", a), ("b1b", b)):
                im[nm] = np.ascontiguousarray(
                    b1[e].reshape(2, MP, P).transpose(2, 0, 1)
                )
        if has_b2:
            for nm, e in (("b2a", a), ("b2b", b)):
                im[nm] = np.ascontiguousarray(b2[e].reshape(DP, P).T)
        in_maps.append(im)

    res = run_bass_kernel_spmd(nc, in_maps, list(range(NCORES)))

    out = np.zeros((N, D), dtype=np.float32)
    for core, (a, ta, ga, b, tb, gb) in enumerate(core_slices):
        # y is [P, DP, C] = YT[d % 128, d // 128, t]; undo the transpose and
        # apply the gates host-side
        yt = res.results[core]["y"]
        y2 = yt.transpose(2, 1, 0).reshape(-1, D)
        out[ta] += ga[:, None] * y2[: len(ta)]
        out[tb] += gb[:, None] * y2[CA : CA + len(tb)]
    return out.reshape(Bb, T, D)


# revision 18
# speedup vs baseline: 1.0758x; 1.0079x over previous
"""MoE layer (E=8 experts, top-2, swiGLU) on 8 TRN2 NeuronCores.

Expert-PAIR parallelism: experts are sorted by routed-token count and
paired hot-with-cold; each pair is split across two cores, each core
processing half of each expert's tokens. This balances the per-core
capacity to ~(n_hot+n_cold)/2 instead of max_e(n_e) — the per-core
compute is proportional to capacity, so balancing is a direct win.
Per core: region A = [0, CA) tokens of expert a, region B = [CA, C) of
expert b (CA/CB global across cores — SPMD). Both experts' W1/W2 stream
to every core (bf16 halves the traffic; it stays far under the compute
time). Router, gates, and the scatter-add combine run on host.

Device kernel structure (per core, SPMD — identical program, per-core data):
  - xt  [P, KO1, C]  tokens, transposed, natural layout           (resident)
  - w2a/w2b [P, KO2, D]                                           (resident)
  - hT  [P, MP, C]   swiGLU output, transposed (H on partitions)  (resident)
  - W1 of both experts streamed in [P, 2, KO1, 128] tiles, one pair per mp
  GEMM1: H1T[h, t] = sum_k W1[k, h] * X[t, k]  (stationary=W1,  moving=xt)
  GEMM2: YT[d, t]  = sum_h W2[h, d] * hT[h, t] (stationary=W2t, moving=hT)
  Token dim streams as the moving operand in chunks of <=512 (PSUM bank
  limit), equalized so no chunk drops under the ~128-col dispatch floor.
  Y leaves transposed [d, t]; the host combine undoes it.

  Prologue: the DMA queues deliver first bytes only at ~9 us (engine
  preamble), so PE-idle time up to that point is absorbed by a few
  warmup matmuls on an uninitialized (output-unread) scratch tile, and
  mp 0 uses a RAMP chunk table [16, 64, 128, 240, ...] paced against DMA
  arrival, with mp 0 and mp 1 interleaved over those chunks so the DMA
  ramp window gets 2x compute per arrived byte. No PE gap ever exceeds
  ~3 us, so the HAM clock gate stays at 2.4 GHz once warmed.
"""

import math

import numpy as np
import ml_dtypes

import concourse.bacc as bacc
import concourse.bass as bass  # noqa: F401
import concourse.mybir as mybir
import concourse.tile as tile
from concourse.bass_utils import run_bass_kernel_spmd

P = 128
NCORES = 8

f32 = mybir.dt.float32
bf16 = mybir.dt.bfloat16
SILU = mybir.ActivationFunctionType.Silu
ADD = mybir.AluOpType.add

NP_BF16 = ml_dtypes.bfloat16


def _ramp_chunks(C, base=0):
    """mp-0 chunk table: small chunks first so compute starts while the
    DMA queue is still ramping, then near-equal chunks <=512."""
    ramp = [16, 64, 128, 240]
    out = []
    c0 = 0
    for r in ramp:
        if c0 + r > C - 256 and c0 + r != C:
            break
        out.append((base + c0, r))
        c0 += r
    out += _chunks(C - c0, base + c0)
    return out


def _chunks(C, base=0):
    """Moving-dim chunks <=512 covering C, sizes equalized (multiple of 8)
    so no chunk drops under the ~128-col dispatch floor."""
    if C <= 0:
        return []
    n = (C + 511) // 512
    lo = (C // n) // 8 * 8
    out = []
    c0 = 0
    for i in range(n):
        cw = min(512, C - c0 - lo * (n - 1 - i))
        cw = cw if i < n - 1 else C - c0
        out.append((base + c0, cw))
        c0 += cw
    assert c0 == C, (C, out)
    return out


def build_moe_pair_nc(D, H, CA, CB, has_b1=False, has_b2=False):
    """Build the SPMD per-expert-pair kernel. D % 128 == 0, H % 128 == 0,
    CA % 4 == CB % 4 == 0 required."""
    C = CA + CB
    KO1 = D // P       # k tiles of GEMM1 (contraction over D)
    MP = H // P        # hidden tiles (per swiGLU half)
    KO2 = H // P       # k tiles of GEMM2 (contraction over H)
    DP = D // P        # GEMM2 output tiles over D

    steadyA = _chunks(CA)
    steadyB = _chunks(CB, CA)
    # the very last GEMM2 chunk is kept small so the final PSUM-drain ->
    # SBUF-copy -> y-DMA pipeline after the last matmul is short
    lc0, lcw = steadyB[-1]
    tail_chunks = steadyA + steadyB[:-1] + (
        [(lc0, lcw - 128), (lc0 + lcw - 128, 128)] if lcw > 256 else [(lc0, lcw)]
    )

    nc = bacc.Bacc(None)
    xt_d = nc.declare_dram_parameter("xt", [P, KO1, C], bf16, isOutput=False)
    w1a_d = nc.declare_dram_parameter("w1a", [MP, P, 2, KO1, P], bf16, isOutput=False)
    w1b_d = nc.declare_dram_parameter("w1b", [MP, P, 2, KO1, P], bf16, isOutput=False)
    w2a_d = nc.declare_dram_parameter("w2a", [P, KO2, D], bf16, isOutput=False)
    w2b_d = nc.declare_dram_parameter("w2b", [P, KO2, D], bf16, isOutput=False)
    if has_b1:
        b1a_d = nc.declare_dram_parameter("b1a", [P, 2, MP], f32, isOutput=False)
        b1b_d = nc.declare_dram_parameter("b1b", [P, 2, MP], f32, isOutput=False)
    if has_b2:
        b2a_d = nc.declare_dram_parameter("b2a", [P, DP], f32, isOutput=False)
        b2b_d = nc.declare_dram_parameter("b2b", [P, DP], f32, isOutput=False)
    y_d = nc.declare_dram_parameter("y", [P, DP, C], f32, isOutput=True)

    with tile.TileContext(nc) as tc:
        with (
            tc.tile_pool(name="sb", bufs=1) as sb,
            tc.tile_pool(name="ps", bufs=1, space="PSUM") as ps,
        ):
            xt_sb = sb.tile([P, KO1, C], bf16)
            w2a_sb = sb.tile([P, KO2, D], bf16)
            w2b_sb = sb.tile([P, KO2, D], bf16)
            hT = sb.tile([P, MP, C], bf16)
            if has_b1:
                b1a_sb = sb.tile([P, 2, MP], f32)
                b1b_sb = sb.tile([P, 2, MP], f32)
            if has_b2:
                b2a_sb = sb.tile([P, DP], f32)
                b2b_sb = sb.tile([P, DP], f32)

            # PE warmup on a DVE-zeroed scratch tile: runs back-to-back from
            # PE-ready (~8 us) past first-data (~9.4 us) until the DMA ramp
            # has landed the first weight tile + token chunk (~14.5 us), and
            # accumulates the >=3.4 us of CONTIGUOUS PE activity the HAM
            # clock gate needs to unthrottle (fires ~11.6 us) — so real
            # matmuls start warm AND data-fed, with no gap ever re-throttling
            # the clock to 1.2 GHz.
            warm = sb.tile([P, 640], bf16)
            nc.vector.memset(warm[:].bitcast(f32), 0.0)
            for wi in range(10):
                warm_ps = ps.tile([P, 512], f32, tag=f"g1_{wi % 6}",
                                  name=f"warm_ps{wi}")
                nc.tensor.matmul(
                    warm_ps[:],
                    lhsT=warm[:, :128],
                    rhs=warm[:, 128:640],
                    start=True,
                    stop=True,
                )

            # ---- prologue loads on the single in-order sync queue, in
            # critical-path order: first matmul group's operands first.
            # mp 0 AND mp 1 run interleaved over the chunks (below), so both
            # mps' weight tiles stream here, pieced between xt chunks.
            w1ta0 = sb.tile([P, 2, KO1, P], bf16, tag="w1ta", bufs=3, name="w1ta0")
            w1ta1 = sb.tile([P, 2, KO1, P], bf16, tag="w1ta", bufs=3, name="w1ta1")
            w1tb0 = sb.tile([P, 2, KO1, P], bf16, tag="w1tb", bufs=3, name="w1tb0")
            w1tb1 = sb.tile([P, 2, KO1, P], bf16, tag="w1tb", bufs=3, name="w1tb1")
            nc.sync.dma_start(w1ta0[:], w1a_d[0])
            pro_chunks = steadyA + steadyB
            pro_iter = iter(pro_chunks)
            for c0, cw in [next(pro_iter)]:
                nc.sync.dma_start(xt_sb[:, :, c0 : c0 + cw], xt_d[:, :, c0 : c0 + cw])
            nc.sync.dma_start(w1ta1[:], w1a_d[1])
            for ci, (c0, cw) in enumerate(pro_iter):
                nc.sync.dma_start(xt_sb[:, :, c0 : c0 + cw], xt_d[:, :, c0 : c0 + cw])
                if ci == 0:
                    # both w1b tiles land before the interleave reaches B
                    nc.sync.dma_start(w1tb0[:], w1b_d[0])
                if ci == 1:
                    nc.sync.dma_start(w1tb1[:], w1b_d[1])
            if has_b1:
                nc.sync.dma_start(b1a_sb[:], b1a_d[:])
                nc.sync.dma_start(b1b_sb[:], b1b_d[:])
            if has_b2:
                nc.sync.dma_start(b2a_sb[:], b2a_d[:])
                nc.sync.dma_start(b2b_sb[:], b2b_d[:])

            def swiglu(psa, psb, cw, mp, c0, b1_sb):
                sg = sb.tile([P, 512], f32, tag="sg", bufs=2)
                if b1_sb is not None:
                    av = sb.tile([P, 512], f32, tag="av", bufs=2)
                    nc.vector.tensor_scalar_add(
                        av[:, :cw], psa[:, :cw], b1_sb[:, 0, mp : mp + 1]
                    )
                    nc.scalar.activation(sg[:, :cw], av[:, :cw], SILU)
                    bs = sb.tile([P, 512], f32, tag="bs", bufs=2)
                    nc.vector.tensor_scalar_add(
                        bs[:, :cw], psb[:, :cw], b1_sb[:, 1, mp : mp + 1]
                    )
                    nc.vector.tensor_mul(
                        hT[:, mp, c0 : c0 + cw], sg[:, :cw], bs[:, :cw]
                    )
                else:
                    nc.scalar.activation(sg[:, :cw], psa[:, :cw], SILU)
                    nc.vector.tensor_mul(
                        hT[:, mp, c0 : c0 + cw], sg[:, :cw], psb[:, :cw]
                    )

            # ---- GEMM1 + swiGLU ----
            ic = 0

            def g1_group(w1t, mp, c0, cw, b1s):
                nonlocal ic
                psa = ps.tile([P, 512], f32, tag=f"g1_{(2 * ic) % 6}")
                psb = ps.tile([P, 512], f32, tag=f"g1_{(2 * ic + 1) % 6}")
                ic += 1
                for k in range(KO1):
                    nc.tensor.matmul(
                        psa[:, :cw],
                        lhsT=w1t[:, 0, k, :],
                        rhs=xt_sb[:, k, c0 : c0 + cw],
                        start=(k == 0),
                        stop=(k == KO1 - 1),
                    )
                for k in range(KO1):
                    nc.tensor.matmul(
                        psb[:, :cw],
                        lhsT=w1t[:, 1, k, :],
                        rhs=xt_sb[:, k, c0 : c0 + cw],
                        start=(k == 0),
                        stop=(k == KO1 - 1),
                    )
                swiglu(psa, psb, cw, mp, c0, b1s)

            b1a = b1a_sb if has_b1 else None
            b1b = b1b_sb if has_b1 else None

            # mps 0 and 1 interleave over the chunks: in the DMA-ramp window,
            # doubling the compute per arrived xt byte keeps the PE fed (no
            # >3 us gap -> no HAM re-throttle) at zero added work.
            for c0, cw in steadyA:
                g1_group(w1ta0, 0, c0, cw, b1a)
                g1_group(w1ta1, 1, c0, cw, b1a)
            for c0, cw in steadyB:
                g1_group(w1tb0, 0, c0, cw, b1b)
                g1_group(w1tb1, 1, c0, cw, b1b)

            for mp in range(2, MP):
                w1ta = sb.tile([P, 2, KO1, P], bf16, tag="w1ta", bufs=3)
                nc.sync.dma_start(w1ta[:], w1a_d[mp])
                w1tb = sb.tile([P, 2, KO1, P], bf16, tag="w1tb", bufs=3)
                nc.sync.dma_start(w1tb[:], w1b_d[mp])
                for c0, cw in steadyA:
                    g1_group(w1ta, mp, c0, cw, b1a)
                for c0, cw in steadyB:
                    g1_group(w1tb, mp, c0, cw, b1b)

            # w2 of both experts ride the queues behind the last w1 tiles
            # (arrive ~40 us before GEMM2 needs them)
            nc.sync.dma_start(w2a_sb[:], w2a_d[:])
            nc.sync.dma_start(w2b_sb[:], w2b_d[:])

            # ---- GEMM2: YT[d, t] — stationary w2 tile, moving hT ----
            iy = 0
            for dp in range(DP):
                tblB = steadyB if dp < DP - 1 else tail_chunks[len(steadyA):]
                for w2sb, tbl, b2s in (
                    (w2a_sb, steadyA, b2a_sb if has_b2 else None),
                    (w2b_sb, tblB, b2b_sb if has_b2 else None),
                ):
                    for c0, cw in tbl:
                        psy = ps.tile([P, 512], f32, tag=f"psy{iy % 2}")
                        iy += 1
                        for k in range(KO2):
                            nc.tensor.matmul(
                                psy[:, :cw],
                                lhsT=w2sb[:, k, dp * P : (dp + 1) * P],
                                rhs=hT[:, k, c0 : c0 + cw],
                                start=(k == 0),
                                stop=(k == KO2 - 1),
                            )
                        ysb = sb.tile([P, 512], f32, tag="ysb", bufs=2)
                        if b2s is not None:
                            nc.vector.tensor_scalar_add(
                                ysb[:, :cw], psy[:, :cw], b2s[:, dp : dp + 1]
                            )
                        else:
                            nc.vector.tensor_copy(ysb[:, :cw], psy[:, :cw])
                        nc.sync.dma_start(y_d[:, dp, c0 : c0 + cw], ysb[:, :cw])
    # run_bass_via_pjrt (the axon execute path) takes a prebuilt module and
    # never finalizes it; Bacc defers register allocation to finalize().
    nc.finalize()
    return nc


def _route(x2, Wr):
    """Top-2 router, numpy fp32 (mirrors jax.lax.top_k + softmax)."""
    n = x2.shape[0]
    ar = np.arange(n)
    z = x2 @ Wr  # [N, E] fp32
    idx1 = z.argmax(axis=1)
    v1 = z[ar, idx1]
    z2 = z.copy()
    z2[ar, idx1] = -np.inf
    idx2 = z2.argmax(axis=1)
    v2 = z2[ar, idx2]
    m = np.maximum(v1, v2)
    e1 = np.exp(v1 - m)
    e2 = np.exp(v2 - m)
    s = e1 + e2
    return idx1, idx2, (e1 / s).astype(np.float32), (e2 / s).astype(np.float32)


def kernel(x, Wr, W1, b1, W2, b2):
    x = np.asarray(x, dtype=np.float32)
    Wr = np.asarray(Wr, dtype=np.float32)
    W1 = np.asarray(W1, dtype=np.float32)
    b1 = np.asarray(b1, dtype=np.float32)
    W2 = np.asarray(W2, dtype=np.float32)
    b2 = np.asarray(b2, dtype=np.float32)

    Bb, T, D = x.shape
    E, _, H2 = W1.shape
    H = H2 // 2
    N = Bb * T
    assert E == NCORES

    x2 = x.reshape(N, D)
    idx1, idx2, g1, g2 = _route(x2, Wr)

    tok = np.concatenate([np.arange(N), np.arange(N)])
    exp = np.concatenate([idx1, idx2])
    gat = np.concatenate([g1, g2])

    toks_e = [tok[exp == e] for e in range(E)]
    gats_e = [gat[exp == e] for e in range(E)]
    counts = np.array([len(t) for t in toks_e])

    # hot-with-cold expert pairing; each pair splits across two cores
    order = np.argsort(-counts)
    pairs = [(int(order[i]), int(order[E - 1 - i])) for i in range(E // 2)]
    CA = max(512, math.ceil(max((counts[a] + 1) // 2 for a, _ in pairs) / 4) * 4)
    CB = max(512, math.ceil(max((counts[b] + 1) // 2 for _, b in pairs) / 4) * 4)
    C = CA + CB

    has_b1 = bool(np.any(b1))
    has_b2 = bool(np.any(b2))

    nc = build_moe_pair_nc(D, H, CA, CB, has_b1=has_b1, has_b2=has_b2)

    KO1 = D // P
    MP = H // P
    KO2 = H // P
    DP = D // P

    # per-core token slices: core 2i gets the first halves of pair i,
    # core 2i+1 the second halves
    core_slices = []   # (a, a_toks, a_gats, b, b_toks, b_gats)
    for a, b in pairs:
        ta, ga = toks_e[a], gats_e[a]
        tb, gb = toks_e[b], gats_e[b]
        ha, hb = (len(ta) + 1) // 2, (len(tb) + 1) // 2
        core_slices.append((a, ta[:ha], ga[:ha], b, tb[:hb], gb[:hb]))
        core_slices.append((a, ta[ha:], ga[ha:], b, tb[hb:], gb[hb:]))

    w1_t = [
        np.ascontiguousarray(
            W1[e].astype(NP_BF16).reshape(KO1, P, 2, MP, P).transpose(3, 1, 2, 0, 4)
        )
        for e in range(E)
    ]
    w2_t = [
        np.ascontiguousarray(
            W2[e].astype(NP_BF16).reshape(KO2, P, D).transpose(1, 0, 2)
        )
        for e in range(E)
    ]

    in_maps = []
    for a, ta, ga, b, tb, gb in core_slices:
        xtf = np.zeros((D, C), dtype=NP_BF16)
        xtf[:, : len(ta)] = x2[ta].astype(NP_BF16).T
        xtf[:, CA : CA + len(tb)] = x2[tb].astype(NP_BF16).T
        xt_t = np.ascontiguousarray(xtf.reshape(KO1, P, C).transpose(1, 0, 2))

        im = {"xt": xt_t, "w1a": w1_t[a], "w1b": w1_t[b],
              "w2a": w2_t[a], "w2b": w2_t[b]}
        if has_b1:
            for nm, e in (("b1a", a), ("b1b", b)):
                im[nm] = np.ascontiguousarray(
                    b1[e].reshape(2, MP, P).transpose(2, 0, 1)
                )
        if has_b2:
            for nm, e in (("b2a", a), ("b2b", b)):
                im[nm] = np.ascontiguousarray(b2[e].reshape(DP, P).T)
        in_maps.append(im)

    res = run_bass_kernel_spmd(nc, in_maps, list(range(NCORES)))

    out = np.zeros((N, D), dtype=np.float32)
    for core, (a, ta, ga, b, tb, gb) in enumerate(core_slices):
        # y is [P, DP, C] = YT[d % 128, d // 128, t]; undo the transpose and
        # apply the gates host-side
        yt = res.results[core]["y"]
        y2 = yt.transpose(2, 1, 0).reshape(-1, D)
        out[ta] += ga[:, None] * y2[: len(ta)]
        out[tb] += gb[:, None] * y2[CA : CA + len(tb)]
    return out.reshape(Bb, T, D)


# revision 20
# speedup vs baseline: 1.0769x; 1.0011x over previous
"""MoE layer (E=8 experts, top-2, swiGLU) on 8 TRN2 NeuronCores.

Expert-PAIR parallelism: experts are sorted by routed-token count and
paired hot-with-cold; each pair is split across two cores, each core
processing half of each expert's tokens. This balances the per-core
capacity to ~(n_hot+n_cold)/2 instead of max_e(n_e) — the per-core
compute is proportional to capacity, so balancing is a direct win.
Per core: region A = [0, CA) tokens of expert a, region B = [CA, C) of
expert b (CA/CB global across cores — SPMD). Both experts' W1/W2 stream
to every core (bf16 halves the traffic; it stays far under the compute
time). Router, gates, and the scatter-add combine run on host.

Device kernel structure (per core, SPMD — identical program, per-core data):
  - xt  [P, KO1, C]  tokens, transposed, natural layout           (resident)
  - w2a/w2b [P, KO2, D]                                           (resident)
  - hT  [P, MP, C]   swiGLU output, transposed (H on partitions)  (resident)
  - W1 of both experts streamed in [P, 2, KO1, 128] tiles, one pair per mp
  GEMM1: H1T[h, t] = sum_k W1[k, h] * X[t, k]  (stationary=W1,  moving=xt)
  GEMM2: YT[d, t]  = sum_h W2[h, d] * hT[h, t] (stationary=W2t, moving=hT)
  Token dim streams as the moving operand in chunks of <=512 (PSUM bank
  limit), equalized so no chunk drops under the ~128-col dispatch floor.
  Y leaves transposed [d, t]; the host combine undoes it.

  Prologue: the DMA queues deliver first bytes only at ~9 us (engine
  preamble), so PE-idle time up to that point is absorbed by a few
  warmup matmuls on an uninitialized (output-unread) scratch tile, and
  mp 0 uses a RAMP chunk table [16, 64, 128, 240, ...] paced against DMA
  arrival, with mp 0 and mp 1 interleaved over those chunks so the DMA
  ramp window gets 2x compute per arrived byte. No PE gap ever exceeds
  ~3 us, so the HAM clock gate stays at 2.4 GHz once warmed.
"""

import math

import numpy as np
import ml_dtypes

import concourse.bacc as bacc
import concourse.bass as bass  # noqa: F401
import concourse.mybir as mybir
import concourse.tile as tile
from concourse.bass_utils import run_bass_kernel_spmd

P = 128
NCORES = 8

f32 = mybir.dt.float32
bf16 = mybir.dt.bfloat16
SILU = mybir.ActivationFunctionType.Silu
ADD = mybir.AluOpType.add

NP_BF16 = ml_dtypes.bfloat16


def _ramp_chunks(C, base=0):
    """mp-0 chunk table: small chunks first so compute starts while the
    DMA queue is still ramping, then near-equal chunks <=512."""
    ramp = [16, 64, 128, 240]
    out = []
    c0 = 0
    for r in ramp:
        if c0 + r > C - 256 and c0 + r != C:
            break
        out.append((base + c0, r))
        c0 += r
    out += _chunks(C - c0, base + c0)
    return out


def _chunks(C, base=0):
    """Moving-dim chunks <=512 covering C, sizes equalized (multiple of 8)
    so no chunk drops under the ~128-col dispatch floor."""
    if C <= 0:
        return []
    n = (C + 511) // 512
    lo = (C // n) // 8 * 8
    out = []
    c0 = 0
    for i in range(n):
        cw = min(512, C - c0 - lo * (n - 1 - i))
        cw = cw if i < n - 1 else C - c0
        out.append((base + c0, cw))
        c0 += cw
    assert c0 == C, (C, out)
    return out


def build_moe_pair_nc(D, H, CA, CB, has_b1=False, has_b2=False):
    """Build the SPMD per-expert-pair kernel. D % 128 == 0, H % 128 == 0,
    CA % 4 == CB % 4 == 0 required."""
    C = CA + CB
    KO1 = D // P       # k tiles of GEMM1 (contraction over D)
    MP = H // P        # hidden tiles (per swiGLU half)
    KO2 = H // P       # k tiles of GEMM2 (contraction over H)
    DP = D // P        # GEMM2 output tiles over D

    steadyA = _chunks(CA)
    steadyB = _chunks(CB, CA)
    # the very last GEMM2 chunk is kept small so the final PSUM-drain ->
    # SBUF-copy -> y-DMA pipeline after the last matmul is short
    lc0, lcw = steadyB[-1]
    tail_chunks = steadyA + steadyB[:-1] + (
        [(lc0, lcw - 128), (lc0 + lcw - 128, 128)] if lcw > 256 else [(lc0, lcw)]
    )

    nc = bacc.Bacc(None)
    xt_d = nc.declare_dram_parameter("xt", [P, KO1, C], bf16, isOutput=False)
    w1a_d = nc.declare_dram_parameter("w1a", [MP, P, 2, KO1, P], bf16, isOutput=False)
    w1b_d = nc.declare_dram_parameter("w1b", [MP, P, 2, KO1, P], bf16, isOutput=False)
    w2a_d = nc.declare_dram_parameter("w2a", [P, KO2, D], bf16, isOutput=False)
    w2b_d = nc.declare_dram_parameter("w2b", [P, KO2, D], bf16, isOutput=False)
    if has_b1:
        b1a_d = nc.declare_dram_parameter("b1a", [P, 2, MP], f32, isOutput=False)
        b1b_d = nc.declare_dram_parameter("b1b", [P, 2, MP], f32, isOutput=False)
    if has_b2:
        b2a_d = nc.declare_dram_parameter("b2a", [P, DP], f32, isOutput=False)
        b2b_d = nc.declare_dram_parameter("b2b", [P, DP], f32, isOutput=False)
    y_d = nc.declare_dram_parameter("y", [P, DP, C], f32, isOutput=True)

    with tile.TileContext(nc) as tc:
        with (
            tc.tile_pool(name="sb", bufs=1) as sb,
            tc.tile_pool(name="ps", bufs=1, space="PSUM") as ps,
        ):
            xt_sb = sb.tile([P, KO1, C], bf16)
            w2a_sb = sb.tile([P, KO2, D], bf16)
            w2b_sb = sb.tile([P, KO2, D], bf16)
            hT = sb.tile([P, MP, C], bf16)
            if has_b1:
                b1a_sb = sb.tile([P, 2, MP], f32)
                b1b_sb = sb.tile([P, 2, MP], f32)
            if has_b2:
                b2a_sb = sb.tile([P, DP], f32)
                b2b_sb = sb.tile([P, DP], f32)

            # PE warmup on a DVE-zeroed scratch tile: runs back-to-back from
            # PE-ready (~8 us) past first-data (~9.4 us) until the DMA ramp
            # has landed the first weight tile + token chunk (~14.5 us), and
            # accumulates the >=3.4 us of CONTIGUOUS PE activity the HAM
            # clock gate needs to unthrottle (fires ~11.6 us) — so real
            # matmuls start warm AND data-fed, with no gap ever re-throttling
            # the clock to 1.2 GHz.
            warm = sb.tile([P, 640], bf16)
            nc.vector.memset(warm[:].bitcast(f32), 0.0)
            for wi in range(11):
                warm_ps = ps.tile([P, 512], f32, tag=f"g1_{wi % 6}",
                                  name=f"warm_ps{wi}")
                nc.tensor.matmul(
                    warm_ps[:],
                    lhsT=warm[:, :128],
                    rhs=warm[:, 128:640],
                    start=True,
                    stop=True,
                )

            # ---- prologue loads on the single in-order sync queue, in
            # critical-path order: first matmul group's operands first.
            # mp 0 AND mp 1 run interleaved over the chunks (below), so both
            # mps' weight tiles stream here, pieced between xt chunks.
            w1ta0 = sb.tile([P, 2, KO1, P], bf16, tag="w1ta", bufs=3, name="w1ta0")
            w1ta1 = sb.tile([P, 2, KO1, P], bf16, tag="w1ta", bufs=3, name="w1ta1")
            w1tb0 = sb.tile([P, 2, KO1, P], bf16, tag="w1tb", bufs=3, name="w1tb0")
            w1tb1 = sb.tile([P, 2, KO1, P], bf16, tag="w1tb", bufs=3, name="w1tb1")
            nc.sync.dma_start(w1ta0[:, 0], w1a_d[0, :, 0])
            pro_chunks = steadyA + steadyB
            pro_iter = iter(pro_chunks)
            for c0, cw in [next(pro_iter)]:
                nc.sync.dma_start(xt_sb[:, :, c0 : c0 + cw], xt_d[:, :, c0 : c0 + cw])
            nc.sync.dma_start(w1ta0[:, 1], w1a_d[0, :, 1])
            nc.sync.dma_start(w1ta1[:], w1a_d[1])
            for ci, (c0, cw) in enumerate(pro_iter):
                nc.sync.dma_start(xt_sb[:, :, c0 : c0 + cw], xt_d[:, :, c0 : c0 + cw])
                if ci == 0:
                    # w1b tile 0 lands before the interleave reaches B...
                    nc.sync.dma_start(w1tb0[:], w1b_d[0])
                if ci == 2:
                    # ...but tile 1 queues after chunk B0 (needed later than
                    # B0, which would otherwise miss its deadline on cores
                    # with a slow DMA ramp)
                    nc.sync.dma_start(w1tb1[:], w1b_d[1])
            if has_b1:
                nc.sync.dma_start(b1a_sb[:], b1a_d[:])
                nc.sync.dma_start(b1b_sb[:], b1b_d[:])
            if has_b2:
                nc.sync.dma_start(b2a_sb[:], b2a_d[:])
                nc.sync.dma_start(b2b_sb[:], b2b_d[:])

            def swiglu(psa, psb, cw, mp, c0, b1_sb):
                sg = sb.tile([P, 512], f32, tag="sg", bufs=2)
                if b1_sb is not None:
                    av = sb.tile([P, 512], f32, tag="av", bufs=2)
                    nc.vector.tensor_scalar_add(
                        av[:, :cw], psa[:, :cw], b1_sb[:, 0, mp : mp + 1]
                    )
                    nc.scalar.activation(sg[:, :cw], av[:, :cw], SILU)
                    bs = sb.tile([P, 512], f32, tag="bs", bufs=2)
                    nc.vector.tensor_scalar_add(
                        bs[:, :cw], psb[:, :cw], b1_sb[:, 1, mp : mp + 1]
                    )
                    nc.vector.tensor_mul(
                        hT[:, mp, c0 : c0 + cw], sg[:, :cw], bs[:, :cw]
                    )
                else:
                    nc.scalar.activation(sg[:, :cw], psa[:, :cw], SILU)
                    nc.vector.tensor_mul(
                        hT[:, mp, c0 : c0 + cw], sg[:, :cw], psb[:, :cw]
                    )

            # ---- GEMM1 + swiGLU ----
            ic = 0

            def g1_group(w1t, mp, c0, cw, b1s):
                nonlocal ic
                psa = ps.tile([P, 512], f32, tag=f"g1_{(2 * ic) % 6}")
                psb = ps.tile([P, 512], f32, tag=f"g1_{(2 * ic + 1) % 6}")
                ic += 1
                for k in range(KO1):
                    nc.tensor.matmul(
                        psa[:, :cw],
                        lhsT=w1t[:, 0, k, :],
                        rhs=xt_sb[:, k, c0 : c0 + cw],
                        start=(k == 0),
                        stop=(k == KO1 - 1),
                    )
                for k in range(KO1):
                    nc.tensor.matmul(
                        psb[:, :cw],
                        lhsT=w1t[:, 1, k, :],
                        rhs=xt_sb[:, k, c0 : c0 + cw],
                        start=(k == 0),
                        stop=(k == KO1 - 1),
                    )
                swiglu(psa, psb, cw, mp, c0, b1s)

            b1a = b1a_sb if has_b1 else None
            b1b = b1b_sb if has_b1 else None

            # mps 0 and 1 interleave over the chunks: in the DMA-ramp window,
            # doubling the compute per arrived xt byte keeps the PE fed (no
            # >3 us gap -> no HAM re-throttle) at zero added work.
            for c0, cw in steadyA:
                g1_group(w1ta0, 0, c0, cw, b1a)
                g1_group(w1ta1, 1, c0, cw, b1a)
            for c0, cw in steadyB:
                g1_group(w1tb0, 0, c0, cw, b1b)
                g1_group(w1tb1, 1, c0, cw, b1b)

            for mp in range(2, MP):
                w1ta = sb.tile([P, 2, KO1, P], bf16, tag="w1ta", bufs=3)
                nc.sync.dma_start(w1ta[:], w1a_d[mp])
                w1tb = sb.tile([P, 2, KO1, P], bf16, tag="w1tb", bufs=3)
                nc.sync.dma_start(w1tb[:], w1b_d[mp])
                for c0, cw in steadyA:
                    g1_group(w1ta, mp, c0, cw, b1a)
                for c0, cw in steadyB:
                    g1_group(w1tb, mp, c0, cw, b1b)

            # w2 of both experts ride the queues behind the last w1 tiles
            # (arrive ~40 us before GEMM2 needs them)
            nc.sync.dma_start(w2a_sb[:], w2a_d[:])
            nc.sync.dma_start(w2b_sb[:], w2b_d[:])

            # ---- GEMM2: YT[d, t] — stationary w2 tile, moving hT ----
            iy = 0
            for dp in range(DP):
                tblB = steadyB if dp < DP - 1 else tail_chunks[len(steadyA):]
                for w2sb, tbl, b2s in (
                    (w2a_sb, steadyA, b2a_sb if has_b2 else None),
                    (w2b_sb, tblB, b2b_sb if has_b2 else None),
                ):
                    for c0, cw in tbl:
                        psy = ps.tile([P, 512], f32, tag=f"psy{iy % 2}")
                        iy += 1
                        for k in range(KO2):
                            nc.tensor.matmul(
                                psy[:, :cw],
                                lhsT=w2sb[:, k, dp * P : (dp + 1) * P],
                                rhs=hT[:, k, c0 : c0 + cw],
                                start=(k == 0),
                                stop=(k == KO2 - 1),
                            )
                        ysb = sb.tile([P, 512], f32, tag="ysb", bufs=2)
                        if b2s is not None:
                            nc.vector.tensor_scalar_add(
                                ysb[:, :cw], psy[:, :cw], b2s[:, dp : dp + 1]
                            )
                        else:
                            nc.vector.tensor_copy(ysb[:, :cw], psy[:, :cw])
                        nc.sync.dma_start(y_d[:, dp, c0 : c0 + cw], ysb[:, :cw])
    # run_bass_via_pjrt (the axon execute path) takes a prebuilt module and
    # never finalizes it; Bacc defers register allocation to finalize().
    nc.finalize()
    return nc


def _route(x2, Wr):
    """Top-2 router, numpy fp32 (mirrors jax.lax.top_k + softmax)."""
    n = x2.shape[0]
    ar = np.arange(n)
    z = x2 @ Wr  # [N, E] fp32
    idx1 = z.argmax(axis=1)
    v1 = z[ar, idx1]
    z2 = z.copy()
    z2[ar, idx1] = -np.inf
    idx2 = z2.argmax(axis=1)
    v2 = z2[ar, idx2]
    m = np.maximum(v1, v2)
    e1 = np.exp(v1 - m)
    e2 = np.exp(v2 - m)
    s = e1 + e2
    return idx1, idx2, (e1 / s).astype(np.float32), (e2 / s).astype(np.float32)


def kernel(x, Wr, W1, b1, W2, b2):
    x = np.asarray(x, dtype=np.float32)
    Wr = np.asarray(Wr, dtype=np.float32)
    W1 = np.asarray(W1, dtype=np.float32)
    b1 = np.asarray(b1, dtype=np.float32)
    W2 = np.asarray(W2, dtype=np.float32)
    b2 = np.asarray(b2, dtype=np.float32)

    Bb, T, D = x.shape
    E, _, H2 = W1.shape
    H = H2 // 2
    N = Bb * T
    assert E == NCORES

    x2 = x.reshape(N, D)
    idx1, idx2, g1, g2 = _route(x2, Wr)

    tok = np.concatenate([np.arange(N), np.arange(N)])
    exp = np.concatenate([idx1, idx2])
    gat = np.concatenate([g1, g2])

    toks_e = [tok[exp == e] for e in range(E)]
    gats_e = [gat[exp == e] for e in range(E)]
    counts = np.array([len(t) for t in toks_e])

    # hot-with-cold expert pairing; each pair splits across two cores
    order = np.argsort(-counts)
    pairs = [(int(order[i]), int(order[E - 1 - i])) for i in range(E // 2)]
    CA = max(512, math.ceil(max((counts[a] + 1) // 2 for a, _ in pairs) / 4) * 4)
    CB = max(512, math.ceil(max((counts[b] + 1) // 2 for _, b in pairs) / 4) * 4)
    C = CA + CB

    has_b1 = bool(np.any(b1))
    has_b2 = bool(np.any(b2))

    nc = build_moe_pair_nc(D, H, CA, CB, has_b1=has_b1, has_b2=has_b2)

    KO1 = D // P
    MP = H // P
    KO2 = H // P
    DP = D // P

    # per-core token slices: core 2i gets the first halves of pair i,
    # core 2i+1 the second halves
    core_slices = []   # (a, a_toks, a_gats, b, b_toks, b_gats)
    for a, b in pairs:
        ta, ga = toks_e[a], gats_e[a]
        tb, gb = toks_e[b], gats_e[b]
        ha, hb = (len(ta) + 1) // 2, (len(tb) + 1) // 2
        core_slices.append((a, ta[:ha], ga[:ha], b, tb[:hb], gb[:hb]))
        core_slices.append((a, ta[ha:], ga[ha:], b, tb[hb:], gb[hb:]))

    w1_t = [
        np.ascontiguousarray(
            W1[e].astype(NP_BF16).reshape(KO1, P, 2, MP, P).transpose(3, 1, 2, 0, 4)
        )
        for e in range(E)
    ]
    w2_t = [
        np.ascontiguousarray(
            W2[e].astype(NP_BF16).reshape(KO2, P, D).transpose(1, 0, 2)
        )
        for e in range(E)
    ]

    in_maps = []
    for a, ta, ga, b, tb, gb in core_slices:
        xtf = np.zeros((D, C), dtype=NP_BF16)
        xtf[:, : len(ta)] = x2[ta].astype(NP_BF16).T
        xtf[:, CA : CA + len(tb)] = x2[tb].astype(NP_BF16).T
        xt_t = np.ascontiguousarray(xtf.reshape(KO1, P, C).transpose(1, 0, 2))

        im = {"xt": xt_t, "w1a": w1_t[a], "w1b": w1_t[b],
              "w2a": w2_t[a], "w2b": w2_t[b]}
        if has_b1:
            for nm, e in (("b1a", a), ("b1b", b)):
                im[nm] = np.ascontiguousarray(
                    b1[e].reshape(2, MP, P).transpose(2, 0, 1)
                )
        if has_b2:
            for nm, e in (("b2a", a), ("b2b", b)):
                im[nm] = np.ascontiguousarray(b2[e].reshape(DP, P).T)
        in_maps.append(im)

    res = run_bass_kernel_spmd(nc, in_maps, list(range(NCORES)))

    out = np.zeros((N, D), dtype=np.float32)
    for core, (a, ta, ga, b, tb, gb) in enumerate(core_slices):
        # y is [P, DP, C] = YT[d % 128, d // 128, t]; undo the transpose and
        # apply the gates host-side
        yt = res.results[core]["y"]
        y2 = yt.transpose(2, 1, 0).reshape(-1, D)
        out[ta] += ga[:, None] * y2[: len(ta)]
        out[tb] += gb[:, None] * y2[CA : CA + len(tb)]
    return out.reshape(Bb, T, D)


# revision 23
# speedup vs baseline: 1.0804x; 1.0032x over previous
"""MoE layer (E=8 experts, top-2, swiGLU) on 8 TRN2 NeuronCores.

Expert-PAIR parallelism: experts are sorted by routed-token count and
paired hot-with-cold; each pair is split across two cores, each core
processing half of each expert's tokens. This balances the per-core
capacity to ~(n_hot+n_cold)/2 instead of max_e(n_e) — the per-core
compute is proportional to capacity, so balancing is a direct win.
Per core: region A = [0, CA) tokens of expert a, region B = [CA, C) of
expert b (CA/CB global across cores — SPMD). Both experts' W1/W2 stream
to every core (bf16 halves the traffic; it stays far under the compute
time). Router, gates, and the scatter-add combine run on host.

Device kernel structure (per core, SPMD — identical program, per-core data):
  - xt  [P, KO1, C]  tokens, transposed, natural layout           (resident)
  - w2a/w2b [P, KO2, D]                                           (resident)
  - hT  [P, MP, C]   swiGLU output, transposed (H on partitions)  (resident)
  - W1 of both experts streamed in [P, 2, KO1, 128] tiles, one pair per mp
  GEMM1: H1T[h, t] = sum_k W1[k, h] * X[t, k]  (stationary=W1,  moving=xt)
  GEMM2: YT[d, t]  = sum_h W2[h, d] * hT[h, t] (stationary=W2t, moving=hT)
  Token dim streams as the moving operand in chunks of <=512 (PSUM bank
  limit), equalized so no chunk drops under the ~128-col dispatch floor.
  Y leaves transposed [d, t]; the host combine undoes it.

  Prologue: the DMA queues deliver first bytes only at ~9 us (engine
  preamble), so PE-idle time up to that point is absorbed by a few
  warmup matmuls on an uninitialized (output-unread) scratch tile, and
  mp 0 uses a RAMP chunk table [16, 64, 128, 240, ...] paced against DMA
  arrival, with mp 0 and mp 1 interleaved over those chunks so the DMA
  ramp window gets 2x compute per arrived byte. No PE gap ever exceeds
  ~3 us, so the HAM clock gate stays at 2.4 GHz once warmed.
"""

import math

import numpy as np
import ml_dtypes

import concourse.bacc as bacc
import concourse.bass as bass  # noqa: F401
import concourse.mybir as mybir
import concourse.tile as tile
from concourse.bass_utils import run_bass_kernel_spmd

P = 128
NCORES = 8

f32 = mybir.dt.float32
bf16 = mybir.dt.bfloat16
SILU = mybir.ActivationFunctionType.Silu
ADD = mybir.AluOpType.add

NP_BF16 = ml_dtypes.bfloat16


def _ramp_chunks(C, base=0):
    """mp-0 chunk table: small chunks first so compute starts while the
    DMA queue is still ramping, then near-equal chunks <=512."""
    ramp = [16, 64, 128, 240]
    out = []
    c0 = 0
    for r in ramp:
        if c0 + r > C - 256 and c0 + r != C:
            break
        out.append((base + c0, r))
        c0 += r
    out += _chunks(C - c0, base + c0)
    return out


def _chunks(C, base=0):
    """Moving-dim chunks <=512 covering C, sizes equalized (multiple of 8)
    so no chunk drops under the ~128-col dispatch floor."""
    if C <= 0:
        return []
    n = (C + 511) // 512
    lo = (C // n) // 8 * 8
    out = []
    c0 = 0
    for i in range(n):
        cw = min(512, C - c0 - lo * (n - 1 - i))
        cw = cw if i < n - 1 else C - c0
        out.append((base + c0, cw))
        c0 += cw
    assert c0 == C, (C, out)
    return out


def build_moe_pair_nc(D, H, CA, CB, has_b1=False, has_b2=False):
    """Build the SPMD per-expert-pair kernel. D % 128 == 0, H % 128 == 0,
    CA % 4 == CB % 4 == 0 required."""
    C = CA + CB
    KO1 = D // P       # k tiles of GEMM1 (contraction over D)
    MP = H // P        # hidden tiles (per swiGLU half)
    KO2 = H // P       # k tiles of GEMM2 (contraction over H)
    DP = D // P        # GEMM2 output tiles over D

    steadyA = _chunks(CA)
    steadyB = _chunks(CB, CA)
    # the very last GEMM2 chunk is kept small so the final PSUM-drain ->
    # SBUF-copy -> y-DMA pipeline after the last matmul is short
    lc0, lcw = steadyB[-1]
    tail_chunks = steadyA + steadyB[:-1] + (
        [(lc0, lcw - 128), (lc0 + lcw - 128, 128)] if lcw > 256 else [(lc0, lcw)]
    )

    nc = bacc.Bacc(None)
    xt_d = nc.declare_dram_parameter("xt", [P, KO1, C], bf16, isOutput=False)
    w1a_d = nc.declare_dram_parameter("w1a", [MP, P, 2, KO1, P], bf16, isOutput=False)
    w1b_d = nc.declare_dram_parameter("w1b", [MP, P, 2, KO1, P], bf16, isOutput=False)
    w2a_d = nc.declare_dram_parameter("w2a", [P, KO2, D], bf16, isOutput=False)
    w2b_d = nc.declare_dram_parameter("w2b", [P, KO2, D], bf16, isOutput=False)
    if has_b1:
        b1a_d = nc.declare_dram_parameter("b1a", [P, 2, MP], f32, isOutput=False)
        b1b_d = nc.declare_dram_parameter("b1b", [P, 2, MP], f32, isOutput=False)
    if has_b2:
        b2a_d = nc.declare_dram_parameter("b2a", [P, DP], f32, isOutput=False)
        b2b_d = nc.declare_dram_parameter("b2b", [P, DP], f32, isOutput=False)
    y_d = nc.declare_dram_parameter("y", [P, DP, C], f32, isOutput=True)

    with tile.TileContext(nc) as tc:
        with (
            tc.tile_pool(name="sb", bufs=1) as sb,
            tc.tile_pool(name="ps", bufs=1, space="PSUM") as ps,
        ):
            xt_sb = sb.tile([P, KO1, C], bf16)
            w2a_sb = sb.tile([P, KO2, D], bf16)
            w2b_sb = sb.tile([P, KO2, D], bf16)
            hT = sb.tile([P, MP, C], bf16)
            if has_b1:
                b1a_sb = sb.tile([P, 2, MP], f32)
                b1b_sb = sb.tile([P, 2, MP], f32)
            if has_b2:
                b2a_sb = sb.tile([P, DP], f32)
                b2b_sb = sb.tile([P, DP], f32)

            # PE warmup on a DVE-zeroed scratch tile: runs back-to-back from
            # PE-ready (~8 us) past first-data (~9.4 us) until the DMA ramp
            # has landed the first weight tile + token chunk (~14.5 us), and
            # accumulates the >=3.4 us of CONTIGUOUS PE activity the HAM
            # clock gate needs to unthrottle (fires ~11.6 us) — so real
            # matmuls start warm AND data-fed, with no gap ever re-throttling
            # the clock to 1.2 GHz.
            warm = sb.tile([P, 640], bf16)
            nc.vector.memset(warm[:].bitcast(f32), 0.0)
            for wi in range(9):
                warm_ps = ps.tile([P, 512], f32, tag=f"g1_{wi % 6}",
                                  name=f"warm_ps{wi}")
                nc.tensor.matmul(
                    warm_ps[:],
                    lhsT=warm[:, :128],
                    rhs=warm[:, 128:640],
                    start=True,
                    stop=True,
                )

            # ---- prologue loads on the single in-order sync queue, in
            # critical-path order: first matmul group's operands first.
            # mp 0 AND mp 1 run interleaved over the chunks (below), so both
            # mps' weight tiles stream here, pieced between xt chunks.
            w1ta0 = sb.tile([P, 2, KO1, P], bf16, tag="w1ta", bufs=3, name="w1ta0")
            w1ta1 = sb.tile([P, 2, KO1, P], bf16, tag="w1ta", bufs=3, name="w1ta1")
            w1tb0 = sb.tile([P, 2, KO1, P], bf16, tag="w1tb", bufs=3, name="w1tb0")
            w1tb1 = sb.tile([P, 2, KO1, P], bf16, tag="w1tb", bufs=3, name="w1tb1")
            # the interleave phase (mps 0+1) splits chunk A0 in half so the
            # very first matmul group only gates on ~0.6 MB of DMA
            a00, a0w = steadyA[0]
            h0w = a0w // 2 // 2 * 2
            phaseA = [(a00, h0w), (a00 + h0w, a0w - h0w)] + steadyA[1:]
            nc.sync.dma_start(w1ta0[:, 0], w1a_d[0, :, 0])
            nc.sync.dma_start(
                xt_sb[:, :, a00 : a00 + h0w], xt_d[:, :, a00 : a00 + h0w]
            )
            nc.sync.dma_start(w1ta0[:, 1], w1a_d[0, :, 1])
            nc.sync.dma_start(
                xt_sb[:, :, a00 + h0w : a00 + a0w],
                xt_d[:, :, a00 + h0w : a00 + a0w],
            )
            nc.sync.dma_start(w1ta1[:], w1a_d[1])
            pro_iter = iter(steadyA[1:] + steadyB)
            for ci, (c0, cw) in enumerate(pro_iter):
                nc.sync.dma_start(xt_sb[:, :, c0 : c0 + cw], xt_d[:, :, c0 : c0 + cw])
                if ci == 0:
                    # w1b tile 0 lands before the interleave reaches B...
                    nc.sync.dma_start(w1tb0[:], w1b_d[0])
                if ci == 2:
                    # ...but tile 1 queues after chunk B0 (needed later than
                    # B0, which would otherwise miss its deadline on cores
                    # with a slow DMA ramp)
                    nc.sync.dma_start(w1tb1[:], w1b_d[1])
            if has_b1:
                nc.sync.dma_start(b1a_sb[:], b1a_d[:])
                nc.sync.dma_start(b1b_sb[:], b1b_d[:])
            if has_b2:
                nc.sync.dma_start(b2a_sb[:], b2a_d[:])
                nc.sync.dma_start(b2b_sb[:], b2b_d[:])

            def swiglu(psa, psb, cw, mp, c0, b1_sb):
                sg = sb.tile([P, 512], f32, tag="sg", bufs=2)
                if b1_sb is not None:
                    av = sb.tile([P, 512], f32, tag="av", bufs=2)
                    nc.vector.tensor_scalar_add(
                        av[:, :cw], psa[:, :cw], b1_sb[:, 0, mp : mp + 1]
                    )
                    nc.scalar.activation(sg[:, :cw], av[:, :cw], SILU)
                    bs = sb.tile([P, 512], f32, tag="bs", bufs=2)
                    nc.vector.tensor_scalar_add(
                        bs[:, :cw], psb[:, :cw], b1_sb[:, 1, mp : mp + 1]
                    )
                    nc.vector.tensor_mul(
                        hT[:, mp, c0 : c0 + cw], sg[:, :cw], bs[:, :cw]
                    )
                else:
                    nc.scalar.activation(sg[:, :cw], psa[:, :cw], SILU)
                    nc.vector.tensor_mul(
                        hT[:, mp, c0 : c0 + cw], sg[:, :cw], psb[:, :cw]
                    )

            # ---- GEMM1 + swiGLU ----
            ic = 0

            def g1_group(w1t, mp, c0, cw, b1s):
                nonlocal ic
                psa = ps.tile([P, 512], f32, tag=f"g1_{(2 * ic) % 6}")
                psb = ps.tile([P, 512], f32, tag=f"g1_{(2 * ic + 1) % 6}")
                ic += 1
                for k in range(KO1):
                    nc.tensor.matmul(
                        psa[:, :cw],
                        lhsT=w1t[:, 0, k, :],
                        rhs=xt_sb[:, k, c0 : c0 + cw],
                        start=(k == 0),
                        stop=(k == KO1 - 1),
                    )
                for k in range(KO1):
                    nc.tensor.matmul(
                        psb[:, :cw],
                        lhsT=w1t[:, 1, k, :],
                        rhs=xt_sb[:, k, c0 : c0 + cw],
                        start=(k == 0),
                        stop=(k == KO1 - 1),
                    )
                swiglu(psa, psb, cw, mp, c0, b1s)

            b1a = b1a_sb if has_b1 else None
            b1b = b1b_sb if has_b1 else None

            # mps 0 and 1 interleave over the chunks: in the DMA-ramp window,
            # doubling the compute per arrived xt byte keeps the PE fed (no
            # >3 us gap -> no HAM re-throttle) at zero added work.
            for c0, cw in phaseA:
                g1_group(w1ta0, 0, c0, cw, b1a)
                g1_group(w1ta1, 1, c0, cw, b1a)
            for c0, cw in steadyB:
                g1_group(w1tb0, 0, c0, cw, b1b)
                g1_group(w1tb1, 1, c0, cw, b1b)

            for mp in range(2, MP):
                w1ta = sb.tile([P, 2, KO1, P], bf16, tag="w1ta", bufs=3)
                nc.sync.dma_start(w1ta[:], w1a_d[mp])
                w1tb = sb.tile([P, 2, KO1, P], bf16, tag="w1tb", bufs=3)
                nc.sync.dma_start(w1tb[:], w1b_d[mp])
                for c0, cw in steadyA:
                    g1_group(w1ta, mp, c0, cw, b1a)
                for c0, cw in steadyB:
                    g1_group(w1tb, mp, c0, cw, b1b)

            # w2 of both experts ride the queues behind the last w1 tiles
            # (arrive ~40 us before GEMM2 needs them)
            nc.sync.dma_start(w2a_sb[:], w2a_d[:])
            nc.sync.dma_start(w2b_sb[:], w2b_d[:])

            # ---- GEMM2: YT[d, t] — stationary w2 tile, moving hT ----
            iy = 0
            for dp in range(DP):
                tblB = steadyB if dp < DP - 1 else tail_chunks[len(steadyA):]
                for w2sb, tbl, b2s in (
                    (w2a_sb, steadyA, b2a_sb if has_b2 else None),
                    (w2b_sb, tblB, b2b_sb if has_b2 else None),
                ):
                    for c0, cw in tbl:
                        psy = ps.tile([P, 512], f32, tag=f"psy{iy % 2}")
                        iy += 1
                        for k in range(KO2):
                            nc.tensor.matmul(
                                psy[:, :cw],
                                lhsT=w2sb[:, k, dp * P : (dp + 1) * P],
                                rhs=hT[:, k, c0 : c0 + cw],
                                start=(k == 0),
                                stop=(k == KO2 - 1),
                            )
                        ysb = sb.tile([P, 512], f32, tag="ysb", bufs=2)
                        if b2s is not None:
                            nc.vector.tensor_scalar_add(
                                ysb[:, :cw], psy[:, :cw], b2s[:, dp : dp + 1]
                            )
                        else:
                            nc.vector.tensor_copy(ysb[:, :cw], psy[:, :cw])
                        nc.sync.dma_start(y_d[:, dp, c0 : c0 + cw], ysb[:, :cw])
    # run_bass_via_pjrt (the axon execute path) takes a prebuilt module and
    # never finalizes it; Bacc defers register allocation to finalize().
    nc.finalize()
    return nc


def _route(x2, Wr):
    """Top-2 router, numpy fp32 (mirrors jax.lax.top_k + softmax)."""
    n = x2.shape[0]
    ar = np.arange(n)
    z = x2 @ Wr  # [N, E] fp32
    idx1 = z.argmax(axis=1)
    v1 = z[ar, idx1]
    z2 = z.copy()
    z2[ar, idx1] = -np.inf
    idx2 = z2.argmax(axis=1)
    v2 = z2[ar, idx2]
    m = np.maximum(v1, v2)
    e1 = np.exp(v1 - m)
    e2 = np.exp(v2 - m)
    s = e1 + e2
    return idx1, idx2, (e1 / s).astype(np.float32), (e2 / s).astype(np.float32)


def kernel(x, Wr, W1, b1, W2, b2):
    x = np.asarray(x, dtype=np.float32)
    Wr = np.asarray(Wr, dtype=np.float32)
    W1 = np.asarray(W1, dtype=np.float32)
    b1 = np.asarray(b1, dtype=np.float32)
    W2 = np.asarray(W2, dtype=np.float32)
    b2 = np.asarray(b2, dtype=np.float32)

    Bb, T, D = x.shape
    E, _, H2 = W1.shape
    H = H2 // 2
    N = Bb * T
    assert E == NCORES

    x2 = x.reshape(N, D)
    idx1, idx2, g1, g2 = _route(x2, Wr)

    tok = np.concatenate([np.arange(N), np.arange(N)])
    exp = np.concatenate([idx1, idx2])
    gat = np.concatenate([g1, g2])

    toks_e = [tok[exp == e] for e in range(E)]
    gats_e = [gat[exp == e] for e in range(E)]
    counts = np.array([len(t) for t in toks_e])

    # hot-with-cold expert pairing; each pair splits across two cores
    order = np.argsort(-counts)
    pairs = [(int(order[i]), int(order[E - 1 - i])) for i in range(E // 2)]
    CA = max(512, math.ceil(max((counts[a] + 1) // 2 for a, _ in pairs) / 4) * 4)
    CB = max(512, math.ceil(max((counts[b] + 1) // 2 for _, b in pairs) / 4) * 4)
    C = CA + CB

    has_b1 = bool(np.any(b1))
    has_b2 = bool(np.any(b2))

    nc = build_moe_pair_nc(D, H, CA, CB, has_b1=has_b1, has_b2=has_b2)

    KO1 = D // P
    MP = H // P
    KO2 = H // P
    DP = D // P

    # per-core token slices: core 2i gets the first halves of pair i,
    # core 2i+1 the second halves
    core_slices = []   # (a, a_toks, a_gats, b, b_toks, b_gats)
    for a, b in pairs:
        ta, ga = toks_e[a], gats_e[a]
        tb, gb = toks_e[b], gats_e[b]
        ha, hb = (len(ta) + 1) // 2, (len(tb) + 1) // 2
        core_slices.append((a, ta[:ha], ga[:ha], b, tb[:hb], gb[:hb]))
        core_slices.append((a, ta[ha:], ga[ha:], b, tb[hb:], gb[hb:]))

    w1_t = [
        np.ascontiguousarray(
            W1[e].astype(NP_BF16).reshape(KO1, P, 2, MP, P).transpose(3, 1, 2, 0, 4)
        )
        for e in range(E)
    ]
    w2_t = [
        np.ascontiguousarray(
            W2[e].astype(NP_BF16).reshape(KO2, P, D).transpose(1, 0, 2)
        )
        for e in range(E)
    ]

    in_maps = []
    for a, ta, ga, b, tb, gb in core_slices:
        xtf = np.zeros((D, C), dtype=NP_BF16)
        xtf[:, : len(ta)] = x2[ta].astype(NP_BF16).T
        xtf[:, CA : CA + len(tb)] = x2[tb].astype(NP_BF16).T
        xt_t = np.ascontiguousarray(xtf.reshape(KO1, P, C).transpose(1, 0, 2))

        im = {"xt": xt_t, "w1a": w1_t[a], "w1b": w1_t[b],
              "w2a": w2_t[a], "w2b": w2_t[b]}
        if has_b1:
            for nm, e in (("b1a", a), ("b1b", b)):
                im[nm] = np.ascontiguousarray(
                    b1[e].reshape(2, MP, P).transpose(2, 0, 1)
                )
        if has_b2:
            for nm, e in (("b2a", a), ("b2b", b)):
                im[nm] = np.ascontiguousarray(b2[e].reshape(DP, P).T)
        in_maps.append(im)

    res = run_bass_kernel_spmd(nc, in_maps, list(range(NCORES)))

    out = np.zeros((N, D), dtype=np.float32)
    for core, (a, ta, ga, b, tb, gb) in enumerate(core_slices):
        # y is [P, DP, C] = YT[d % 128, d // 128, t]; undo the transpose and
        # apply the gates host-side
        yt = res.results[core]["y"]
        y2 = yt.transpose(2, 1, 0).reshape(-1, D)
        out[ta] += ga[:, None] * y2[: len(ta)]
        out[tb] += gb[:, None] * y2[CA : CA + len(tb)]
    return out.reshape(Bb, T, D)
